# revision 6
# baseline (speedup 1.0000x reference)
"""GCN embedder kernel for TRN2, 8-core SPMD (v6: 128-wide dst windows,
pure-one-hot scatter matmuls, dinv factoring, unified 3-layer edge pass).

Design
------
* Nodes sharded contiguously across C=8 cores (NC=12500 each). Edges are
  owned by the dst core. Self-loops are NOT materialized as edges.
* Normalization factored: norm(s,d) = dinv[s]*dinv[d]. dinv[src] is
  pre-multiplied into the gather table rows (T'[v] = dinv[v] * (h@W)[v]),
  dinv[dst] is applied at PSUM flush. The self-loop term
  dinv[d]^2*(h@W)[d] = dinv[d]*T'[d] is injected by an identity matmul
  of the local feature-major T' into PSUM before the edge matmuls.
* Edge pass (identical structure for all 3 layers): edges sorted by
  (src_chunk, window, dst) where window = 128 dst nodes. Per-edge-tile
  (128 edges) the one-hot B[e, drel] = is_equal(iota, drel) is ONE DVE
  tensor_scalar (or 2 scalar-engine ACTs for a fraction of tiles); one
  PE matmul accumulates into the 512-wide quad PSUM bank at the window's
  128-col offset. Layers 1-2: lhsT=messages, rhs=B -> feature-major PSUM.
  Layer 3: lhsT=B, rhs=messages -> node-major PSUM (pooling needs
  node-major and layer 3 feeds nothing else).
* Gathers: per (chunk, quad-range) - ~100 calls/layer of ~2-3K indices
  (994ns fixed GpSimd cost per SWDGE call makes small calls ruinous).
  Table rows are bf16 [N,128]=256B. Indices int16 (chunk-relative).
  The SAME index/meta streams serve all 3 layers (layer 1's table is
  per-node: T1n[v] = dinv[v]*T1[x[v]], vocab-gathered on chip).
* Table phase (interleaved into the edge pass of the previous layer at
  quad granularity): hwT = W^T @ hT (PE), T'T = hwT * dinvR (DVE),
  transpose blocks to rows (PE), DMA to DRAM shard; AllGather.
* Pooling: layer-3 node-major blocks feed is_equal(batchrel) one-hot
  matmuls directly (lhsT=Bpool half, rhs=h3 block) accumulating
  [grel<=256, H] in 2 PSUM banks; flush rows scatter by graph id
  (indirect DMA), AllReduce, scale by 1/cnt.

All structure (tile counts, call sizes) is maxed across cores so the
single SPMD program fits every core; pad slots have drel=-1 (zero
one-hot column) and index 0 (valid row).
"""

import math
from contextlib import ExitStack
from dataclasses import dataclass, field

import numpy as np

import concourse.mybir as mybir
import concourse.tile as tile
from concourse import bacc, bass
from concourse.bass import AP, IndirectOffsetOnAxis, ds
from concourse.masks import make_identity

F32 = mybir.dt.float32
BF16 = mybir.dt.bfloat16
I16 = mybir.dt.int16
I32 = mybir.dt.int32
AF = mybir.ActivationFunctionType
OP = mybir.AluOpType

P = 128  # partitions / hidden size / vocab

DEBUG_STAGE = 0  # 0=off; 1..3 = dump h after that layer


@dataclass
class Cfg:
    N: int = 100000
    E: int = 1600000
    H: int = 128
    V: int = 128
    L: int = 3
    G: int = 1024
    C: int = 8          # cores
    CH: int = 4         # gather-table chunks (int16 index limit)
    WIN: int = 128      # dst window width
    NQ: int = 4         # SWDGE queues
    SCALAR_FRAC_NUM: int = 1   # of every DEN tiles, NUM one-hots on Scalar
    SCALAR_FRAC_DEN: int = 7

    @property
    def NC(self):
        assert self.N % self.C == 0
        return self.N // self.C

    @property
    def CHN(self):
        assert self.N % self.CH == 0
        return self.N // self.CH

    @property
    def W(self):  # 128-wide dst windows per core
        return math.ceil(self.NC / self.WIN)

    @property
    def Q(self):  # 512-wide quads (ranges) per core
        return math.ceil(self.W / 4)

    @property
    def NCP(self):
        return self.W * self.WIN

    @property
    def GSPAN(self):
        return 256


@dataclass
class Structure:
    # t_cw[c][w]: edge tiles for (chunk c, window w), maxed across cores
    t_cw: list = field(default_factory=list)

    @property
    def T(self):
        return sum(sum(r) for r in self.t_cw)


def preprocess(x, edge_index, batch, emb_table, Ws, bs, cfg: Cfg):
    """Host-side (index-only) preprocessing."""
    N, E, C, CH = cfg.N, cfg.E, cfg.C, cfg.CH
    NC, CHN, W, Q, WIN = cfg.NC, cfg.CHN, cfg.W, cfg.Q, cfg.WIN

    x = np.asarray(x).astype(np.int64)
    edge_index = np.asarray(edge_index).astype(np.int64)
    batch = np.asarray(batch).astype(np.int64)

    src, dst = edge_index[0], edge_index[1]
    deg = (np.bincount(dst, minlength=N) + 1).astype(np.float32)  # + self
    dinv = (1.0 / np.sqrt(deg)).astype(np.float32)

    owner = dst // NC
    per_core = []
    for c in range(C):
        m = owner == c
        s_c = src[m]
        d_c = dst[m] - c * NC
        w_c = d_c // WIN
        ck = s_c // CHN
        o = np.lexsort((d_c, w_c, ck))  # chunk-major, then window, then dst
        per_core.append(dict(s=s_c[o] % CHN, drel=(d_c[o] % WIN),
                             grp=(ck[o] * W + w_c[o])))

    # tiles per (chunk, window), maxed across cores
    t_cw = np.zeros((CH, W), dtype=np.int64)
    for c in range(C):
        cnt = np.bincount(per_core[c]["grp"], minlength=CH * W)
        t_cw = np.maximum(t_cw, -(-cnt.reshape(CH, W) // P))
    assert (t_cw >= 1).all()
    st = Structure(t_cw=[list(map(int, r)) for r in t_cw])
    T = st.T

    # global tile offset of group (c, w) in the (chunk-major) stream
    gtile = np.zeros((CH, W), dtype=np.int64)
    off = 0
    for c in range(CH):
        for w in range(W):
            gtile[c, w] = off
            off += t_cw[c, w]
    assert off == T

    # gather calls: one per (chunk, quad-range of 4 windows)
    # call (c, q) covers tiles gtile[c, 4q] .. (+ nt(c,q))
    nt_cq = np.zeros((CH, Q), dtype=np.int64)
    for c in range(CH):
        for q in range(Q):
            nt_cq[c, q] = t_cw[c, 4 * q:4 * q + 4].sum()
    NTC = int(nt_cq.max())

    def wrap(arr):
        # [n] int16 -> [128, n//16] wrapped in 16 partitions, tiled x8
        wr = arr.reshape(-1, 16).T
        return np.tile(wr, (8, 1))

    in_maps = []
    for c in range(C):
        pc = per_core[c]
        cnt = np.bincount(pc["grp"], minlength=CH * W).reshape(CH, W)
        starts = np.zeros((CH, W), dtype=np.int64)
        pos = 0
        for cc in range(CH):
            for w in range(W):
                starts[cc, w] = pos
                pos += cnt[cc, w]

        meta = np.full((P, T), -1.0, dtype=np.float32)
        idxs = np.zeros(T * P, dtype=np.int16)
        for cc in range(CH):
            for w in range(W):
                n = int(cnt[cc, w])
                sl = slice(int(starts[cc, w]), int(starts[cc, w]) + n)
                t0 = int(gtile[cc, w])
                ii = np.arange(n)
                meta[ii % P, t0 + ii // P] = pc["drel"][sl]
                idxs[t0 * P + ii] = pc["s"][sl].astype(np.int16)

        # gidx in range-major call order: [q][c] concatenated
        gidx = np.zeros((P, T * 8), dtype=np.int16)
        col = 0
        for q in range(Q):
            for cc in range(CH):
                t0, nt = int(gtile[cc, 4 * q]), int(nt_cq[cc, q])
                gidx[:, col:col + nt * 8] = wrap(idxs[t0 * P:(t0 + nt) * P])
                col += nt * 8
        assert col == T * 8

        # vocab gather indices for the layer-1 table (node-major, pad->0)
        nodes = np.arange(cfg.NCP) + c * NC
        valid = nodes < (c + 1) * NC
        xl = np.where(valid, x[np.minimum(nodes, N - 1)], 0)
        xidx = wrap(xl.astype(np.int16))  # [128, NCP//16*8] = [128, W*64]

        dloc = np.where(valid, dinv[np.minimum(nodes, N - 1)], 0.0)
        dinv_pm = dloc.reshape(W, P).T.copy().astype(np.float32)  # [128, W]
        dinvR = np.broadcast_to(dloc[None, :], (P, cfg.NCP)).astype(np.float32)

        bias3R = np.broadcast_to(np.asarray(bs)[2][None, :],
                                 (P, cfg.H)).astype(np.float32).copy()

        # pooling metadata (as baseline)
        bvals = np.where(valid, batch[np.minimum(nodes, N - 1)], -1)
        gmin = int(batch[c * NC])
        gmax = int(batch[min((c + 1) * NC, N) - 1])
        assert gmax - gmin < cfg.GSPAN, (c, gmin, gmax)
        brel = np.where(valid, bvals - gmin, -1).astype(np.float32)
        pool_meta = brel.reshape(W, P).T.copy()  # [128, W]
        gid_rows = gmin + np.arange(cfg.GSPAN)
        gid_rows = np.where(gid_rows < cfg.G, gid_rows,
                            cfg.G + np.arange(cfg.GSPAN) % 256).astype(np.int32)
        gid_cols = gid_rows.reshape(2, P).T.copy()  # [128, 2]

        cnts = np.bincount(batch, minlength=cfg.G).astype(np.float32)
        recip = 1.0 / np.maximum(cnts, 1.0)
        recip_pm = recip.reshape(cfg.G // P, P).T.copy()

        in_maps.append({
            "meta": meta, "gidx": gidx, "xidx": np.ascontiguousarray(xidx),
            "dinv_pm": dinv_pm, "dinvR": np.ascontiguousarray(dinvR),
            "bias3R": bias3R,
            "pool_meta": pool_meta, "gid_cols": gid_cols,
            "recip_pm": recip_pm,
            "emb": np.asarray(emb_table, dtype=np.float32),
            "Ws": np.asarray(Ws, dtype=np.float32),
            "bs": np.asarray(bs, dtype=np.float32),
        })

    st.nt_cq = [list(map(int, r)) for r in nt_cq]
    st.gtile = [list(map(int, r)) for r in gtile]
    st.NTC = NTC
    return st, in_maps


# --------------------------------------------------------------------------
# device program
# --------------------------------------------------------------------------

def build_nc(cfg: Cfg, st: Structure):
    N, H, C, CH, W, Q = cfg.N, cfg.H, cfg.C, cfg.CH, cfg.W, cfg.Q
    NC, CHN, NCP, WIN = cfg.NC, cfg.CHN, cfg.NCP, cfg.WIN
    T = st.T
    NTC = st.NTC
    GS = cfg.GSPAN
    GW = cfg.G // P
    NQ = cfg.NQ

    nc = bacc.Bacc(None, num_devices=C, num_swdge_queues=NQ)
    cores = list(range(C))

    # ---- external I/O ----
    meta_d = nc.declare_dram_parameter("meta", [P, T], F32, isOutput=False)
    gidx_d = nc.declare_dram_parameter("gidx", [P, T * 8], I16, isOutput=False)
    xidx_d = nc.declare_dram_parameter("xidx", [P, W * 8], I16, isOutput=False)
    dinv_pm_d = nc.declare_dram_parameter("dinv_pm", [P, W], F32, isOutput=False)
    dinvR_d = nc.declare_dram_parameter("dinvR", [P, NCP], F32, isOutput=False)
    bias3R_d = nc.declare_dram_parameter("bias3R", [P, H], F32, isOutput=False)
    pool_meta = nc.declare_dram_parameter("pool_meta", [P, W], F32, isOutput=False)
    gid_cols = nc.declare_dram_parameter("gid_cols", [P, 2], I32, isOutput=False)
    recip_pm = nc.declare_dram_parameter("recip_pm", [P, GW], F32, isOutput=False)
    emb_d = nc.declare_dram_parameter("emb", [P, H], F32, isOutput=False)
    Ws_d = nc.declare_dram_parameter("Ws", [cfg.L, H, H], F32, isOutput=False)
    bs_d = nc.declare_dram_parameter("bs", [cfg.L, H], F32, isOutput=False)
    out_d = nc.declare_dram_parameter("out", [cfg.G, H], F32, isOutput=True)

    # ---- internal DRAM ----
    t1_dram = nc.dram_tensor("t1_tab", [cfg.V, H], BF16)
    tab_shard = nc.dram_tensor("tab_shard", [NC, H], BF16)
    tab_full = nc.dram_tensor("tab_full", [N, H], BF16, addr_space="Shared")
    pooled_nm = nc.dram_tensor("pooled_nm", [cfg.G + GS, H], F32)
    pooled_sum = nc.dram_tensor("pooled_sum", [cfg.G + GS, H], F32,
                                addr_space="Shared")

    from concourse.tile import add_dep_helper
    pd = {"i": 0, "last": None}

    def chain_pool_dma(inst):
        if pd["last"] is not None:
            add_dep_helper(inst.ins, pd["last"].ins, sync=False,
                           reason="pool-dma queue/lane parity order")
        pd["last"] = inst
        pd["i"] += 1

    with tile.TileContext(nc) as tc, ExitStack() as ctx:
        const = ctx.enter_context(tc.tile_pool(name="const", bufs=1))
        hpool = ctx.enter_context(tc.tile_pool(name="hbuf", bufs=1))

        ident = const.tile([P, P], F32)
        make_identity(nc, ident[:])
        ident_bf = const.tile([P, P], BF16)
        make_identity(nc, ident_bf[:])
        iota_i = const.tile([P, 512], I32)
        nc.gpsimd.iota(iota_i[:], pattern=[[1, 512]], base=0,
                       channel_multiplier=0)
        iota_v = const.tile([P, WIN], F32)   # DVE one-hot input
        nc.vector.tensor_copy(out=iota_v[:], in_=iota_i[:, :WIN])
        iota_s = const.tile([P, WIN], F32)   # Scalar one-hot input
        nc.vector.tensor_copy(out=iota_s[:], in_=iota_i[:, :WIN])
        iota_pool = const.tile([P, GS], BF16)
        nc.vector.tensor_copy(out=iota_pool[:], in_=iota_i[:, :GS])

        b_cols = const.tile([P, cfg.L], F32)
        for l in range(cfg.L):
            nc.sync.dma_start(out=b_cols[:, l:l + 1], in_=bs_d[l, :, None])
        w_bf = const.tile([P, cfg.L * H], BF16, tag="w_bf")
        with tc.tile_pool(name="wload", bufs=2) as wl:
            for l in range(cfg.L):
                wt = wl.tile([P, H], F32, tag="wt")
                nc.sync.dma_start(out=wt[:], in_=Ws_d[l])
                nc.vector.tensor_copy(out=w_bf[:, l * H:(l + 1) * H], in_=wt[:])
        bias3R = const.tile([P, H], F32)
        nc.sync.dma_start(out=bias3R[:], in_=bias3R_d[:, :])
        dinv_pm = const.tile([P, W], F32)
        nc.sync.dma_start(out=dinv_pm[:], in_=dinv_pm_d[:, :])

        # resident meta + dinvR (bf16)
        meta_sb = const.tile([P, T], F32, tag="meta_sb")
        MC = 4096
        for s0 in range(0, T, MC):
            nn = min(MC, T - s0)
            nc.sync.dma_start(out=meta_sb[:, s0:s0 + nn],
                              in_=meta_d[:, s0:s0 + nn])
        dinvR = const.tile([P, NCP], BF16, tag="dinvR")
        with tc.tile_pool(name="dld", bufs=2) as dld:
            for s0 in range(0, NCP, 512):
                nn = min(512, NCP - s0)
                dt_ = dld.tile([P, 512], F32, tag="d")
                nc.sync.dma_start(out=dt_[:, :nn], in_=dinvR_d[:, s0:s0 + nn])
                nc.vector.tensor_copy(out=dinvR[:, s0:s0 + nn],
                                      in_=dt_[:, :nn])

        hT_a = hpool.tile([P, NCP], BF16)     # feature-major h (layers 1,2)
        hT_b = hpool.tile([P, NCP], BF16)
        TpT = hpool.tile([P, NCP], BF16)      # feature-major local T'
        h3nm = hT_a                           # layer-3 out (node-major) aliases
        #                                       layer-1 h (dead by then)

        # ---------------- layer-1 table: T1n[v] = dinv[v]*T1[x[v]] ----------
        with nc.named_scope("boot"), \
             tc.tile_pool(name="pro", bufs=2) as pro, \
             tc.tile_pool(name="pro_ps", bufs=2, space="PSUM") as pro_ps, \
             tc.tile_pool(name="bootg", bufs=2) as bootg, \
             tc.tile_pool(name="bootix", bufs=1) as bootix:
            emb_sb = pro.tile([P, H], F32, tag="emb")
            nc.sync.dma_start(out=emb_sb[:], in_=emb_d[:, :])
            w1_sb = pro.tile([P, H], F32, tag="w1")
            nc.sync.dma_start(out=w1_sb[:], in_=Ws_d[0])
            embT_ps = pro_ps.tile([P, P], F32)
            nc.tensor.transpose(out=embT_ps[:], in_=emb_sb[:], identity=ident[:])
            embT = pro.tile([P, P], F32, tag="embT")
            nc.vector.tensor_copy(out=embT[:], in_=embT_ps[:])
            t1t_ps = pro_ps.tile([P, P], F32)
            nc.tensor.matmul(out=t1t_ps[:], lhsT=w1_sb[:], rhs=embT[:],
                             start=True, stop=True)
            t1t = pro.tile([P, P], F32, tag="t1t")
            nc.vector.tensor_copy(out=t1t[:], in_=t1t_ps[:])
            t1nm_ps = pro_ps.tile([P, P], F32)
            nc.tensor.transpose(out=t1nm_ps[:], in_=t1t[:], identity=ident[:])
            t1nm = pro.tile([P, P], BF16, tag="t1nm")
            nc.vector.tensor_copy(out=t1nm[:], in_=t1nm_ps[:])
            nc.sync.dma_start(out=t1_dram[:, :], in_=t1nm[:])

            # vocab gather (node-major), scale by dinv, rows -> tab_shard,
            # transpose -> TpT
            xix = bootix.tile([P, W * 8], I16, tag="xix")
            nc.sync.dma_start(out=xix[:], in_=xidx_d[:, :])
            BG = 14  # tiles per vocab-gather call
            for t0 in range(0, W, BG):
                nt = min(BG, W - t0)
                g = bootg.tile([P, BG, H], BF16, tag="vg")
                gi = nc.gpsimd.dma_gather(
                    out_ap=g[:, :nt, :], in_ap=t1_dram[:, :],
                    idxs_ap=xix[:, t0 * 8:(t0 + nt) * 8],
                    num_idxs=nt * P, num_idxs_reg=nt * P,
                    elem_size=H, single_packet=False,
                    queue_num=pd["i"] % NQ)
                chain_pool_dma(gi)
                for i in range(nt):
                    t = t0 + i
                    nq = min(P, NC - t * P)
                    if nq <= 0:
                        break
                    mn = bootg.tile([P, P], BF16, tag="mn")
                    nc.vector.tensor_scalar(
                        out=mn[:], in0=g[:, i, :],
                        scalar1=dinv_pm[:, t:t + 1], scalar2=None,
                        op0=OP.mult)
                    nc.sync.dma_start(out=tab_shard[t * P:t * P + nq, :],
                                      in_=mn[:nq, :])
                    tp_ps = pro_ps.tile([P, P], BF16, tag="tp")
                    nc.tensor.transpose(out=tp_ps[:], in_=mn[:],
                                        identity=ident_bf[:])
                    nc.scalar.activation(out=TpT[:, t * P:(t + 1) * P],
                                         in_=tp_ps[:], func=AF.Copy)
            nc.gpsimd.collective_compute(
                "AllGather", OP.bypass, replica_groups=[cores],
                ins=[tab_shard[:, :]], outs=[tab_full[:, :]])

        # ---------------- unified edge pass ----------------
        tilectr = [0]
        t_cw = st.t_cw
        nt_cq = st.nt_cq
        gtile = st.gtile

        # gidx column offset of call (q, c) in range-major layout
        gcol = {}
        col = 0
        for q in range(Q):
            for c in range(CH):
                gcol[(q, c)] = col
                col += nt_cq[c][q] * 8
        qcol = {}  # column offset of range q's gidx block, and width
        for q in range(Q):
            qcol[q] = (gcol[(q, 0)],
                       sum(nt_cq[c][q] for c in range(CH)) * 8)

        def edge_pass(layer, h_out, node_major):
            lname = f"layer{layer + 1}"
            with nc.named_scope(lname), \
                 tc.tile_pool(name=f"ix{layer}", bufs=2) as ixp, \
                 tc.tile_pool(name=f"gb{layer}", bufs=2) as gb, \
                 tc.tile_pool(name=f"bq{layer}", bufs=8) as bq, \
                 tc.tile_pool(name=f"fl{layer}", bufs=3) as fl, \
                 tc.tile_pool(name=f"eps{layer}", bufs=3, space="PSUM") as eps, \
                 tc.tile_pool(name=f"tps{layer}", bufs=2, space="PSUM") as tps, \
                 tc.tile_pool(name=f"tps2{layer}", bufs=2, space="PSUM") as tps2, \
                 tc.tile_pool(name=f"tbl{layer}", bufs=3) as tbl:

                gbuf = {}

                def issue_range(q):
                    c0, cw = qcol[q]
                    gx = ixp.tile([P, max(w for _, w in qcol.values())],
                                  I16, tag="gx")
                    nc.sync.dma_start(out=gx[:, :cw],
                                      in_=gidx_d[:, c0:c0 + cw])
                    for c in range(CH):
                        nt = nt_cq[c][q]
                        g = gb.tile([P, NTC, H], BF16, tag=f"g{c}")
                        gi = nc.gpsimd.dma_gather(
                            out_ap=g[:, :nt, :],
                            in_ap=tab_full[c * CHN:(c + 1) * CHN, :],
                            idxs_ap=gx[:, gcol[(q, c)] - c0:
                                       gcol[(q, c)] - c0 + nt * 8],
                            num_idxs=nt * P, num_idxs_reg=nt * P,
                            elem_size=H, single_packet=False,
                            queue_num=pd["i"] % NQ)
                        chain_pool_dma(gi)
                        gbuf[(q, c)] = g

                def emit_tile(g, slot, gt, qpsum, wrel, last):
                    Bt = bq.tile([P, WIN], BF16, tag="B")
                    tc_ = tilectr[0]
                    tilectr[0] += 1
                    den, num = cfg.SCALAR_FRAC_DEN, cfg.SCALAR_FRAC_NUM
                    if tc_ % den < num:
                        a = bq.tile([P, WIN], F32, tag="A")
                        nc.scalar.activation(
                            out=a[:], in_=iota_s[:], func=AF.Abs,
                            bias=meta_sb[:, gt:gt + 1], scale=-1.0)
                        nc.scalar.activation(
                            out=Bt[:], in_=a[:], func=AF.Relu,
                            bias=1.0, scale=-1.0)
                    else:
                        nc.vector.tensor_scalar(
                            out=Bt[:], in0=iota_v[:],
                            scalar1=meta_sb[:, gt:gt + 1], scalar2=None,
                            op0=OP.is_equal)
                    reg = qpsum[:, wrel * WIN:(wrel + 1) * WIN]
                    if node_major:
                        nc.tensor.matmul(out=reg, lhsT=Bt[:],
                                         rhs=g[:, slot, :],
                                         start=False, stop=last)
                    else:
                        nc.tensor.matmul(out=reg, lhsT=g[:, slot, :],
                                         rhs=Bt[:],
                                         start=False, stop=last)

                def table_phase_quad(q, nxt_layer):
                    # hw for quad q of h_out -> T' rows + TpT (for next layer)
                    ncol = min(512, NCP - q * 512)
                    hw_ps = tps.tile([P, 512], F32, tag="hw")
                    nc.tensor.matmul(
                        out=hw_ps[:, :ncol],
                        lhsT=w_bf[:, nxt_layer * H:(nxt_layer + 1) * H],
                        rhs=h_out[:, q * 512:q * 512 + ncol],
                        start=True, stop=True)
                    nc.vector.tensor_tensor(
                        out=TpT[:, q * 512:q * 512 + ncol],
                        in0=hw_ps[:, :ncol],
                        in1=dinvR[:, q * 512:q * 512 + ncol], op=OP.mult)
                    for b in range(4):
                        t = q * 4 + b
                        if t >= W:
                            break
                        nq = min(P, NC - t * P)
                        if nq <= 0:
                            break
                        tp_ps = tps2.tile([P, P], BF16, tag="tr")
                        nc.tensor.transpose(
                            out=tp_ps[:], in_=TpT[:, t * P:(t + 1) * P],
                            identity=ident_bf[:])
                        stg = tbl.tile([P, P], BF16, tag="stg")
                        nc.scalar.activation(out=stg[:], in_=tp_ps[:],
                                             func=AF.Copy)
                        nc.sync.dma_start(
                            out=tab_shard[t * P:t * P + nq, :],
                            in_=stg[:nq, :])

                def pool_quad(q, pool_ps0, pool_ps1, pm, h_src):
                    for b in range(4):
                        t = q * 4 + b
                        if t >= W:
                            break
                        Bp = fl.tile([P, GS], BF16, tag="Bp")
                        nc.vector.tensor_scalar(
                            out=Bp[:], in0=iota_pool[:],
                            scalar1=pm[:, t:t + 1], scalar2=None,
                            op0=OP.is_equal)
                        blk = h_src[:, t * P:(t + 1) * P]
                        nc.tensor.matmul(out=pool_ps0[:], lhsT=Bp[:, :P],
                                         rhs=blk, start=(t == 0),
                                         stop=(t == W - 1))
                        nc.tensor.matmul(out=pool_ps1[:], lhsT=Bp[:, P:],
                                         rhs=blk, start=(t == 0),
                                         stop=(t == W - 1))

                issue_range(0)
                for q in range(Q):
                    if q + 1 < Q:
                        issue_range(q + 1)
                    ncol = min(512, NCP - q * 512)
                    qpsum = eps.tile([P, 512], F32, tag="qp")
                    # self-loop injection (opens the accumulation group)
                    if node_major:
                        for b in range(4):
                            w = q * 4 + b
                            if w >= W:
                                break
                            nc.tensor.matmul(
                                out=qpsum[:, b * WIN:(b + 1) * WIN],
                                lhsT=TpT[:, w * WIN:(w + 1) * WIN],
                                rhs=ident_bf[:], start=(b == 0), stop=False)
                    else:
                        nc.tensor.matmul(
                            out=qpsum[:, :ncol], lhsT=ident_bf[:],
                            rhs=TpT[:, q * 512:q * 512 + ncol],
                            start=True, stop=False)
                    # edge matmuls
                    nq_tiles = sum(t_cw[c][w] for c in range(CH)
                                   for w in range(q * 4, min(q * 4 + 4, W)))
                    done = 0
                    for b in range(4):
                        w = q * 4 + b
                        if w >= W:
                            break
                        for c in range(CH):
                            g = gbuf[(q, c)]
                            base = gtile[c][q * 4]  # first tile of call
                            for i in range(t_cw[c][w]):
                                slot = (gtile[c][w] - base) + i
                                gt = gtile[c][w] + i
                                done += 1
                                emit_tile(g, slot, gt, qpsum, b,
                                          done == nq_tiles)
                    # flush
                    if node_major:
                        for b in range(4):
                            w = q * 4 + b
                            if w >= W:
                                break
                            nc.vector.scalar_tensor_tensor(
                                out=h_out[:, w * WIN:(w + 1) * WIN],
                                in0=qpsum[:, b * WIN:(b + 1) * WIN],
                                scalar=dinv_pm[:, w:w + 1],
                                in1=bias3R[:], op0=OP.mult, op1=OP.add)
                    else:
                        tmp = fl.tile([P, 512], BF16, tag="tmp")
                        nc.vector.tensor_tensor(
                            out=tmp[:, :ncol], in0=qpsum[:, :ncol],
                            in1=dinvR[:, q * 512:q * 512 + ncol], op=OP.mult)
                        nc.scalar.activation(
                            out=h_out[:, q * 512:q * 512 + ncol],
                            in_=tmp[:, :ncol], func=AF.Relu,
                            bias=b_cols[:, layer:layer + 1], scale=1.0)
                    # interleaved next-phase work
                    if layer < cfg.L - 1 and DEBUG_STAGE == 0:
                        table_phase_quad(q, layer + 1)
                if layer < cfg.L - 1:
                    if DEBUG_STAGE != 0:
                        for q in range(Q):
                            table_phase_quad(q, layer + 1)
                    nc.gpsimd.collective_compute(
                        "AllGather", OP.bypass, replica_groups=[cores],
                        ins=[tab_shard[:, :]], outs=[tab_full[:, :]])

        def dump_h(src_tile):
            dbg_d = nc.declare_dram_parameter("dbg", [P, NCP], F32,
                                              isOutput=True)
            with tc.tile_pool(name="dbg", bufs=2) as dbp:
                CWD = 512
                for s0 in range(0, NCP, CWD):
                    nn = min(CWD, NCP - s0)
                    dt_ = dbp.tile([P, CWD], F32, tag="d")
                    nc.vector.tensor_copy(out=dt_[:, :nn],
                                          in_=src_tile[:, s0:s0 + nn])
                    nc.sync.dma_start(out=dbg_d[:, s0:s0 + nn],
                                      in_=dt_[:, :nn])

        edge_pass(0, hT_a, node_major=False)
        if DEBUG_STAGE == 1:
            dump_h(hT_a)
        edge_pass(1, hT_b, node_major=False)
        if DEBUG_STAGE == 2:
            dump_h(hT_b)
        edge_pass(2, h3nm, node_major=True)
        if DEBUG_STAGE == 3:
            dump_h(h3nm)

        # ---------------- pooling ----------------
        with nc.named_scope("pool"), \
             tc.tile_pool(name="po", bufs=3) as po, \
             tc.tile_pool(name="po_ps", bufs=2, space="PSUM") as po_ps, \
             tc.tile_pool(name="po_acc", bufs=2, space="PSUM") as po_acc:
            pm = po.tile([P, W], F32, tag="pm")
            nc.sync.dma_start(out=pm[:], in_=pool_meta[:, :])
            gcols = po.tile([P, 2], I32, tag="gcols")
            nc.sync.dma_start(out=gcols[:], in_=gid_cols[:, :])
            recip_sb = po.tile([P, GW], F32, tag="recip")
            nc.sync.dma_start(out=recip_sb[:], in_=recip_pm[:, :])

            acc0 = po_acc.tile([P, P], F32)
            acc1 = po_acc.tile([P, P], F32)
            for t in range(W):
                Bp = po.tile([P, GS], BF16, tag="Bp")
                nc.vector.tensor_scalar(
                    out=Bp[:], in0=iota_pool[:],
                    scalar1=pm[:, t:t + 1], scalar2=None,
                    op0=OP.is_equal)
                blk = h3nm[:, t * P:(t + 1) * P]
                nc.tensor.matmul(out=acc0[:], lhsT=Bp[:, :P], rhs=blk,
                                 start=(t == 0), stop=(t == W - 1))
                nc.tensor.matmul(out=acc1[:], lhsT=Bp[:, P:], rhs=blk,
                                 start=(t == 0), stop=(t == W - 1))

            def dummy_gather():
                dz = po.tile([P, 1, P], BF16, tag="dz")
                zi = po.tile([P, 8], I16, tag="zi")
                nc.vector.memset(zi[:], 0)
                gi = nc.gpsimd.dma_gather(
                    out_ap=dz[:], in_ap=t1_dram[:, :], idxs_ap=zi[:],
                    num_idxs=P, num_idxs_reg=P, elem_size=H,
                    single_packet=False, queue_num=pd["i"] % NQ)
                chain_pool_dma(gi)

            zt = po.tile([P, P], F32, tag="zt")
            nc.vector.memset(zt[:], 0.0)
            for r0 in range(0, cfg.G + GS, P):
                nc.sync.dma_start(out=pooled_nm[r0:r0 + P, :], in_=zt[:])

            for half, acc in ((0, acc0), (1, acc1)):
                rows = po.tile([P, P], F32, tag="rows")
                nc.scalar.activation(out=rows[:], in_=acc[:], func=AF.Copy)
                while pd["i"] % NQ != 0:
                    dummy_gather()  # scatters run on queue 0: align lane
                si = nc.gpsimd.indirect_dma_start(
                    out=pooled_nm[:, :],
                    out_offset=IndirectOffsetOnAxis(
                        ap=gcols[:, half:half + 1], axis=0),
                    in_=rows[:], in_offset=None)
                chain_pool_dma(si)

            nc.gpsimd.collective_compute(
                "AllReduce", OP.add, replica_groups=[cores],
                ins=[pooled_nm[:, :]], outs=[pooled_sum[:, :]])

            for gw in range(GW):
                ot = po.tile([P, H], F32, tag="ot")
                nc.sync.dma_start(out=ot[:],
                                  in_=pooled_sum[gw * P:(gw + 1) * P, :])
                os = po.tile([P, H], F32, tag="os")
                nc.vector.tensor_scalar(
                    out=os[:], in0=ot[:], scalar1=recip_sb[:, gw:gw + 1],
                    scalar2=None, op0=OP.mult)
                nc.sync.dma_start(out=out_d[gw * P:(gw + 1) * P, :],
                                  in_=os[:])

    return nc


# --------------------------------------------------------------------------
# entry point: full inputs -> full output
# --------------------------------------------------------------------------

_CACHE = {}


def _get_compiled(cfg, st_key, st):
    if st_key not in _CACHE:
        nc = build_nc(cfg, st)
        nc.finalize()
        _CACHE[st_key] = nc
    return _CACHE[st_key]


def kernel(x, edge_index, batch, emb_table, Ws, bs):
    cfg = Cfg()  # full problem size, hardcoded
    st, in_maps = preprocess(x, edge_index, batch, emb_table, Ws, bs, cfg)
    st_key = tuple(tuple(r) for r in st.t_cw)
    nc = _get_compiled(cfg, st_key, st)

    from concourse.bass_utils import run_bass_kernel_spmd

    res = run_bass_kernel_spmd(nc, in_maps, list(range(cfg.C)))
    return np.ascontiguousarray(res.results[0]["out"])


# revision 16
# speedup vs baseline: 1.6901x; 1.6901x over previous
"""GCN embedder kernel for TRN2, 8-core SPMD (v6: 128-wide dst windows,
pure-one-hot scatter matmuls, dinv factoring, unified 3-layer edge pass).

Design
------
* Nodes sharded contiguously across C=8 cores (NC=12500 each). Edges are
  owned by the dst core. Self-loops are NOT materialized as edges.
* Normalization factored: norm(s,d) = dinv[s]*dinv[d]. dinv[src] is
  pre-multiplied into the gather table rows (T'[v] = dinv[v] * (h@W)[v]),
  dinv[dst] is applied at PSUM flush. The self-loop term
  dinv[d]^2*(h@W)[d] = dinv[d]*T'[d] is injected by an identity matmul
  of the local feature-major T' into PSUM before the edge matmuls.
* Edge pass (identical structure for all 3 layers): edges sorted by
  (src_chunk, window, dst) where window = 128 dst nodes. Per-edge-tile
  (128 edges) the one-hot B[e, drel] = is_equal(iota, drel) is ONE DVE
  tensor_scalar (or 2 scalar-engine ACTs for a fraction of tiles); one
  PE matmul accumulates into the 512-wide quad PSUM bank at the window's
  128-col offset. Layers 1-2: lhsT=messages, rhs=B -> feature-major PSUM.
  Layer 3: lhsT=B, rhs=messages -> node-major PSUM (pooling needs
  node-major and layer 3 feeds nothing else).
* Gathers: per (chunk, quad-range) - ~100 calls/layer of ~2-3K indices
  (994ns fixed GpSimd cost per SWDGE call makes small calls ruinous).
  Table rows are bf16 [N,128]=256B. Indices int16 (chunk-relative).
  The SAME index/meta streams serve all 3 layers (layer 1's table is
  per-node: T1n[v] = dinv[v]*T1[x[v]], vocab-gathered on chip).
* Table phase (interleaved into the edge pass of the previous layer at
  quad granularity): hwT = W^T @ hT (PE), T'T = hwT * dinvR (DVE),
  transpose blocks to rows (PE), DMA to DRAM shard; AllGather.
* Pooling: layer-3 node-major blocks feed is_equal(batchrel) one-hot
  matmuls directly (lhsT=Bpool half, rhs=h3 block) accumulating
  [grel<=256, H] in 2 PSUM banks; flush rows scatter by graph id
  (indirect DMA), AllReduce, scale by 1/cnt.

All structure (tile counts, call sizes) is maxed across cores so the
single SPMD program fits every core; pad slots have drel=-1 (zero
one-hot column) and index 0 (valid row).
"""

import math
from contextlib import ExitStack
from dataclasses import dataclass, field

import numpy as np

import concourse.mybir as mybir
import concourse.tile as tile
from concourse import bacc, bass
from concourse.bass import AP, IndirectOffsetOnAxis, ds
from concourse.masks import make_identity

F32 = mybir.dt.float32
BF16 = mybir.dt.bfloat16
I16 = mybir.dt.int16
I32 = mybir.dt.int32
AF = mybir.ActivationFunctionType
OP = mybir.AluOpType

P = 128  # partitions / hidden size / vocab

DEBUG_STAGE = 0  # 0=off; 1..3 = dump h after that layer


@dataclass
class Cfg:
    N: int = 100000
    E: int = 1600000
    H: int = 128
    V: int = 128
    L: int = 3
    G: int = 1024
    C: int = 8          # cores
    CH: int = 4         # gather-table chunks (int16 index limit)
    WIN: int = 128      # dst window width
    NQ: int = 4         # SWDGE queues
    SCALAR_FRAC_NUM: int = 1   # of every DEN tiles, NUM one-hots on Scalar
    SCALAR_FRAC_DEN: int = 7

    @property
    def NC(self):
        assert self.N % self.C == 0
        return self.N // self.C

    @property
    def CHN(self):
        assert self.N % self.CH == 0
        return self.N // self.CH

    @property
    def W(self):  # 128-wide dst windows per core
        return math.ceil(self.NC / self.WIN)

    @property
    def Q(self):  # 512-wide quads (ranges) per core
        return math.ceil(self.W / 4)

    @property
    def NCP(self):
        return self.W * self.WIN

    @property
    def GSPAN(self):
        return 256


@dataclass
class Structure:
    # t_cw[c][w]: edge tiles for (chunk c, window w), maxed across cores
    t_cw: list = field(default_factory=list)

    @property
    def T(self):
        return sum(sum(r) for r in self.t_cw)


def preprocess(x, edge_index, batch, emb_table, Ws, bs, cfg: Cfg):
    """Host-side (index-only) preprocessing."""
    N, E, C, CH = cfg.N, cfg.E, cfg.C, cfg.CH
    NC, CHN, W, Q, WIN = cfg.NC, cfg.CHN, cfg.W, cfg.Q, cfg.WIN

    x = np.asarray(x).astype(np.int64)
    edge_index = np.asarray(edge_index).astype(np.int64)
    batch = np.asarray(batch).astype(np.int64)

    src, dst = edge_index[0], edge_index[1]
    deg = (np.bincount(dst, minlength=N) + 1).astype(np.float32)  # + self
    dinv = (1.0 / np.sqrt(deg)).astype(np.float32)

    owner = dst // NC
    per_core = []
    for c in range(C):
        m = owner == c
        s_c = src[m]
        d_c = dst[m] - c * NC
        w_c = d_c // WIN
        ck = s_c // CHN
        o = np.lexsort((d_c, w_c, ck))  # chunk-major, then window, then dst
        per_core.append(dict(s=s_c[o] % CHN, drel=(d_c[o] % WIN),
                             grp=(ck[o] * W + w_c[o])))

    # tiles per (chunk, window), maxed across cores
    t_cw = np.zeros((CH, W), dtype=np.int64)
    for c in range(C):
        cnt = np.bincount(per_core[c]["grp"], minlength=CH * W)
        t_cw = np.maximum(t_cw, -(-cnt.reshape(CH, W) // P))
    assert (t_cw >= 1).all()
    st = Structure(t_cw=[list(map(int, r)) for r in t_cw])
    T = st.T

    # global tile offset of group (c, w) in the (chunk-major) stream
    gtile = np.zeros((CH, W), dtype=np.int64)
    off = 0
    for c in range(CH):
        for w in range(W):
            gtile[c, w] = off
            off += t_cw[c, w]
    assert off == T

    # gather calls: one per (chunk, quad-range of 4 windows)
    # call (c, q) covers tiles gtile[c, 4q] .. (+ nt(c,q))
    nt_cq = np.zeros((CH, Q), dtype=np.int64)
    for c in range(CH):
        for q in range(Q):
            nt_cq[c, q] = t_cw[c, 4 * q:4 * q + 4].sum()
    NTC = int(nt_cq.max())

    def wrap(arr):
        # [n] int16 -> [128, n//16] wrapped in 16 partitions, tiled x8
        wr = arr.reshape(-1, 16).T
        return np.tile(wr, (8, 1))

    in_maps = []
    for c in range(C):
        pc = per_core[c]
        cnt = np.bincount(pc["grp"], minlength=CH * W).reshape(CH, W)
        starts = np.zeros((CH, W), dtype=np.int64)
        pos = 0
        for cc in range(CH):
            for w in range(W):
                starts[cc, w] = pos
                pos += cnt[cc, w]

        # meta in WINDOW-major (consumption) order: (w, c, i); idx stream in
        # chunk-major (gather) order: (c, w, i)
        meta = np.full((P, T), -1.0, dtype=np.float32)
        idxs = np.zeros(T * P, dtype=np.int16)
        wt0 = 0
        for w in range(W):
            for cc in range(CH):
                n = int(cnt[cc, w])
                sl = slice(int(starts[cc, w]), int(starts[cc, w]) + n)
                ii = np.arange(n)
                meta[ii % P, wt0 + ii // P] = pc["drel"][sl]
                idxs[int(gtile[cc, w]) * P + ii] = pc["s"][sl].astype(np.int16)
                wt0 += int(t_cw[cc, w])
        assert wt0 == T

        # gidx in range-major call order: [q][c] concatenated
        gidx = np.zeros((P, T * 8), dtype=np.int16)
        col = 0
        for q in range(Q):
            for cc in range(CH):
                t0, nt = int(gtile[cc, 4 * q]), int(nt_cq[cc, q])
                gidx[:, col:col + nt * 8] = wrap(idxs[t0 * P:(t0 + nt) * P])
                col += nt * 8
        assert col == T * 8

        # vocab gather indices for the layer-1 table (node-major, pad->0)
        nodes = np.arange(cfg.NCP) + c * NC
        valid = nodes < (c + 1) * NC
        xl = np.where(valid, x[np.minimum(nodes, N - 1)], 0)
        xidx = wrap(xl.astype(np.int16))  # [128, NCP//16*8] = [128, W*64]

        dloc = np.where(valid, dinv[np.minimum(nodes, N - 1)], 0.0)
        dinv_pm = dloc.reshape(W, P).T.copy().astype(np.float32)  # [128, W]
        dinvR = np.broadcast_to(dloc[None, :], (P, cfg.NCP)).astype(np.float32)

        bias3R = np.broadcast_to(np.asarray(bs)[2][None, :],
                                 (P, cfg.H)).astype(np.float32).copy()

        # pooling metadata (as baseline)
        bvals = np.where(valid, batch[np.minimum(nodes, N - 1)], -1)
        gmin = int(batch[c * NC])
        gmax = int(batch[min((c + 1) * NC, N) - 1])
        assert gmax - gmin < cfg.GSPAN, (c, gmin, gmax)
        brel = np.where(valid, bvals - gmin, -1).astype(np.float32)
        pool_meta = brel.reshape(W, P).T.copy()  # [128, W]
        gid_rows = gmin + np.arange(cfg.GSPAN)
        gid_rows = np.where(gid_rows < cfg.G, gid_rows,
                            cfg.G + np.arange(cfg.GSPAN) % 256).astype(np.int32)
        gid_cols = gid_rows.reshape(2, P).T.copy()  # [128, 2]

        cnts = np.bincount(batch, minlength=cfg.G).astype(np.float32)
        recip = 1.0 / np.maximum(cnts, 1.0)
        recip_pm = recip.reshape(cfg.G // P, P).T.copy()

        in_maps.append({
            "meta": meta, "gidx": gidx, "xidx": np.ascontiguousarray(xidx),
            "dinv_pm": dinv_pm, "dinvR": np.ascontiguousarray(dinvR),
            "bias3R": bias3R,
            "pool_meta": pool_meta, "gid_cols": gid_cols,
            "recip_pm": recip_pm,
            "emb": np.asarray(emb_table, dtype=np.float32),
            "Ws": np.asarray(Ws, dtype=np.float32),
            "bs": np.asarray(bs, dtype=np.float32),
        })

    st.nt_cq = [list(map(int, r)) for r in nt_cq]
    st.gtile = [list(map(int, r)) for r in gtile]
    st.NTC = NTC
    st.NTW = int(t_cw.sum(axis=0).max())  # max tiles per window (all chunks)
    return st, in_maps


# --------------------------------------------------------------------------
# device program
# --------------------------------------------------------------------------

def build_nc(cfg: Cfg, st: Structure):
    N, H, C, CH, W, Q = cfg.N, cfg.H, cfg.C, cfg.CH, cfg.W, cfg.Q
    NC, CHN, NCP, WIN = cfg.NC, cfg.CHN, cfg.NCP, cfg.WIN
    T = st.T
    NTC = st.NTC
    GS = cfg.GSPAN
    GW = cfg.G // P
    NQ = cfg.NQ

    nc = bacc.Bacc(None, num_devices=C, num_swdge_queues=NQ)
    cores = list(range(C))

    # ---- external I/O ----
    meta_d = nc.declare_dram_parameter("meta", [P, T], F32, isOutput=False)
    gidx_d = nc.declare_dram_parameter("gidx", [P, T * 8], I16, isOutput=False)
    xidx_d = nc.declare_dram_parameter("xidx", [P, W * 8], I16, isOutput=False)
    dinv_pm_d = nc.declare_dram_parameter("dinv_pm", [P, W], F32, isOutput=False)
    dinvR_d = nc.declare_dram_parameter("dinvR", [P, NCP], F32, isOutput=False)
    bias3R_d = nc.declare_dram_parameter("bias3R", [P, H], F32, isOutput=False)
    pool_meta = nc.declare_dram_parameter("pool_meta", [P, W], F32, isOutput=False)
    gid_cols = nc.declare_dram_parameter("gid_cols", [P, 2], I32, isOutput=False)
    recip_pm = nc.declare_dram_parameter("recip_pm", [P, GW], F32, isOutput=False)
    emb_d = nc.declare_dram_parameter("emb", [P, H], F32, isOutput=False)
    Ws_d = nc.declare_dram_parameter("Ws", [cfg.L, H, H], F32, isOutput=False)
    bs_d = nc.declare_dram_parameter("bs", [cfg.L, H], F32, isOutput=False)
    out_d = nc.declare_dram_parameter("out", [cfg.G, H], F32, isOutput=True)

    # ---- internal DRAM ----
    t1_dram = nc.dram_tensor("t1_tab", [cfg.V, H], BF16)
    tab_shard = nc.dram_tensor("tab_shard", [NC, H], BF16)
    tab_full = nc.dram_tensor("tab_full", [N, H], BF16, addr_space="Shared")
    pooled_nm = nc.dram_tensor("pooled_nm", [cfg.G + GS, H], F32)
    pooled_sum = nc.dram_tensor("pooled_sum", [cfg.G + GS, H], F32,
                                addr_space="Shared")

    from concourse.tile import add_dep_helper
    pd = {"i": 0, "last": None}

    def chain_pool_dma(inst):
        if pd["last"] is not None:
            add_dep_helper(inst.ins, pd["last"].ins, sync=False,
                           reason="pool-dma queue/lane parity order")
        pd["last"] = inst
        pd["i"] += 1

    with tile.TileContext(nc) as tc, ExitStack() as ctx:
        const = ctx.enter_context(tc.tile_pool(name="const", bufs=1))
        hpool = ctx.enter_context(tc.tile_pool(name="hbuf", bufs=1))

        ident = const.tile([P, P], F32)
        make_identity(nc, ident[:])
        ident_bf = const.tile([P, P], BF16)
        make_identity(nc, ident_bf[:])
        iota_i = const.tile([P, 512], I32)
        nc.gpsimd.iota(iota_i[:], pattern=[[1, 512]], base=0,
                       channel_multiplier=0)
        iota_pool = const.tile([P, GS], BF16)
        nc.vector.tensor_copy(out=iota_pool[:], in_=iota_i[:, :GS])
        # repeating 0..WIN-1 pattern, one block per tile of a window group
        NTW = st.NTW
        iotaB_i = const.tile([P, NTW, WIN], I32)
        nc.gpsimd.iota(iotaB_i[:], pattern=[[0, NTW], [1, WIN]], base=0,
                       channel_multiplier=0)
        iotaB = const.tile([P, NTW, WIN], BF16)
        nc.vector.tensor_copy(out=iotaB[:], in_=iotaB_i[:])

        b_cols = const.tile([P, cfg.L], F32)
        for l in range(cfg.L):
            nc.sync.dma_start(out=b_cols[:, l:l + 1], in_=bs_d[l, :, None])
        w_bf = const.tile([P, cfg.L * H], BF16, tag="w_bf")
        with tc.tile_pool(name="wload", bufs=2) as wl:
            for l in range(cfg.L):
                wt = wl.tile([P, H], F32, tag="wt")
                nc.sync.dma_start(out=wt[:], in_=Ws_d[l])
                nc.vector.tensor_copy(out=w_bf[:, l * H:(l + 1) * H], in_=wt[:])
        bias3R = const.tile([P, H], F32)
        nc.sync.dma_start(out=bias3R[:], in_=bias3R_d[:, :])
        dinv_pm = const.tile([P, W], F32)
        nc.sync.dma_start(out=dinv_pm[:], in_=dinv_pm_d[:, :])

        # resident meta (bf16: drel in 0..127 and -1 are exact) + dinvR (bf16)
        meta_bf = const.tile([P, T], BF16, tag="meta_bf")
        with tc.tile_pool(name="mld", bufs=2) as mld:
            MC = 1024
            for s0 in range(0, T, MC):
                nn = min(MC, T - s0)
                mt_ = mld.tile([P, MC], F32, tag="m")
                nc.sync.dma_start(out=mt_[:, :nn], in_=meta_d[:, s0:s0 + nn])
                nc.vector.tensor_copy(out=meta_bf[:, s0:s0 + nn],
                                      in_=mt_[:, :nn])
        dinvR = const.tile([P, NCP], BF16, tag="dinvR")
        with tc.tile_pool(name="dld", bufs=2) as dld:
            for s0 in range(0, NCP, 512):
                nn = min(512, NCP - s0)
                dt_ = dld.tile([P, 512], F32, tag="d")
                nc.sync.dma_start(out=dt_[:, :nn], in_=dinvR_d[:, s0:s0 + nn])
                nc.vector.tensor_copy(out=dinvR[:, s0:s0 + nn],
                                      in_=dt_[:, :nn])

        hT_a = hpool.tile([P, NCP], BF16)     # feature-major h (layers 1,2)
        hT_b = hpool.tile([P, NCP], BF16)
        TpT = hpool.tile([P, NCP], BF16)      # feature-major local T'
        h3nm = hT_a                           # layer-3 out (node-major) aliases
        #                                       layer-1 h (dead by then)

        # ---------------- layer-1 table: T1n[v] = dinv[v]*T1[x[v]] ----------
        with nc.named_scope("boot"), \
             tc.tile_pool(name="pro", bufs=2) as pro, \
             tc.tile_pool(name="pro_ps", bufs=2, space="PSUM") as pro_ps, \
             tc.tile_pool(name="bootg", bufs=2) as bootg, \
             tc.tile_pool(name="bootix", bufs=1) as bootix:
            emb_sb = pro.tile([P, H], F32, tag="emb")
            nc.sync.dma_start(out=emb_sb[:], in_=emb_d[:, :])
            w1_sb = pro.tile([P, H], F32, tag="w1")
            nc.sync.dma_start(out=w1_sb[:], in_=Ws_d[0])
            embT_ps = pro_ps.tile([P, P], F32)
            nc.tensor.transpose(out=embT_ps[:], in_=emb_sb[:], identity=ident[:])
            embT = pro.tile([P, P], F32, tag="embT")
            nc.vector.tensor_copy(out=embT[:], in_=embT_ps[:])
            t1t_ps = pro_ps.tile([P, P], F32)
            nc.tensor.matmul(out=t1t_ps[:], lhsT=w1_sb[:], rhs=embT[:],
                             start=True, stop=True)
            t1t = pro.tile([P, P], F32, tag="t1t")
            nc.vector.tensor_copy(out=t1t[:], in_=t1t_ps[:])
            t1nm_ps = pro_ps.tile([P, P], F32)
            nc.tensor.transpose(out=t1nm_ps[:], in_=t1t[:], identity=ident[:])
            t1nm = pro.tile([P, P], BF16, tag="t1nm")
            nc.vector.tensor_copy(out=t1nm[:], in_=t1nm_ps[:])
            nc.sync.dma_start(out=t1_dram[:, :], in_=t1nm[:])

            # vocab gather (node-major), scale by dinv, rows -> tab_shard,
            # transpose -> TpT
            xix = bootix.tile([P, W * 8], I16, tag="xix")
            nc.sync.dma_start(out=xix[:], in_=xidx_d[:, :])
            BG = 14  # tiles per vocab-gather call
            for t0 in range(0, W, BG):
                nt = min(BG, W - t0)
                g = bootg.tile([P, BG, H], BF16, tag="vg")
                nsub = min(NQ, nt)
                per = -(-nt // nsub)
                s0 = 0
                while s0 < nt:
                    sn = min(per, nt - s0)
                    gi = nc.gpsimd.dma_gather(
                        out_ap=g[:, s0:s0 + sn, :], in_ap=t1_dram[:, :],
                        idxs_ap=xix[:, (t0 + s0) * 8:(t0 + s0 + sn) * 8],
                        num_idxs=sn * P, num_idxs_reg=sn * P,
                        elem_size=H, single_packet=False,
                        queue_num=pd["i"] % NQ)
                    chain_pool_dma(gi)
                    s0 += sn
                # scale whole call's tiles by per-node dinv in one DVE op
                mnb = bootg.tile([P, BG, H], BF16, tag="mn")
                dbase = dinv_pm[:, t0:t0 + nt]
                dbc = bass.AP(dbase.tensor, dbase.offset,
                              list(dbase.ap) + [[0, H]])
                nc.vector.tensor_tensor(out=mnb[:, :nt, :], in0=g[:, :nt, :],
                                        in1=dbc, op=OP.mult)
                for i in range(nt):
                    t = t0 + i
                    nq = min(P, NC - t * P)
                    if nq <= 0:
                        break
                    nc.sync.dma_start(out=tab_shard[t * P:t * P + nq, :],
                                      in_=mnb[:nq, i, :])
                    tp_ps = pro_ps.tile([P, P], BF16, tag="tp")
                    nc.tensor.transpose(out=tp_ps[:], in_=mnb[:, i, :],
                                        identity=ident_bf[:])
                    nc.scalar.activation(out=TpT[:, t * P:(t + 1) * P],
                                         in_=tp_ps[:], func=AF.Copy)
            nc.gpsimd.collective_compute(
                "AllGather", OP.bypass, replica_groups=[cores],
                ins=[tab_shard[:, :]], outs=[tab_full[:, :]])

        # ---------------- unified edge pass ----------------
        tilectr = [0]
        t_cw = st.t_cw
        nt_cq = st.nt_cq
        gtile = st.gtile

        # gidx column offset of call (q, c) in range-major layout
        gcol = {}
        col = 0
        for q in range(Q):
            for c in range(CH):
                gcol[(q, c)] = col
                col += nt_cq[c][q] * 8
        qcol = {}  # column offset of range q's gidx block, and width
        for q in range(Q):
            qcol[q] = (gcol[(q, 0)],
                       sum(nt_cq[c][q] for c in range(CH)) * 8)

        def edge_pass(layer, h_out, node_major):
            lname = f"layer{layer + 1}"
            with nc.named_scope(lname), \
                 tc.tile_pool(name=f"ix{layer}", bufs=2) as ixp, \
                 tc.tile_pool(name=f"gb{layer}", bufs=2) as gb, \
                 tc.tile_pool(name=f"bq{layer}", bufs=3) as bq, \
                 tc.tile_pool(name=f"fl{layer}", bufs=3) as fl, \
                 tc.tile_pool(name=f"eps{layer}", bufs=3, space="PSUM") as eps, \
                 tc.tile_pool(name=f"tps{layer}", bufs=2, space="PSUM") as tps, \
                 tc.tile_pool(name=f"tps2{layer}", bufs=2, space="PSUM") as tps2, \
                 tc.tile_pool(name=f"tbl{layer}", bufs=3) as tbl:

                gbuf = {}

                def issue_range(q):
                    c0, cw = qcol[q]
                    gx = ixp.tile([P, max(w for _, w in qcol.values())],
                                  I16, tag="gx")
                    nc.sync.dma_start(out=gx[:, :cw],
                                      in_=gidx_d[:, c0:c0 + cw])
                    for c in range(CH):
                        nt = nt_cq[c][q]
                        g = gb.tile([P, NTC, H], BF16, tag=f"g{c}")
                        goff = gcol[(q, c)] - c0
                        # split across all queues; batched sub-calls let the
                        # SWDGE ucode generate descriptors on all 4 queue
                        # engines concurrently (~4x effective rate)
                        nsub = min(NQ, nt)
                        per = -(-nt // nsub)
                        s0 = 0
                        while s0 < nt:
                            sn = min(per, nt - s0)
                            gi = nc.gpsimd.dma_gather(
                                out_ap=g[:, s0:s0 + sn, :],
                                in_ap=tab_full[c * CHN:(c + 1) * CHN, :],
                                idxs_ap=gx[:, goff + s0 * 8:
                                           goff + (s0 + sn) * 8],
                                num_idxs=sn * P, num_idxs_reg=sn * P,
                                elem_size=H, single_packet=False,
                                queue_num=pd["i"] % NQ)
                            chain_pool_dma(gi)
                            s0 += sn
                        gbuf[(q, c)] = g

                def build_B(mcol, ntw):
                    # one-hot rows for a whole window group in ONE DVE op:
                    # B[p, t, j] = (iota[j] == drel[p, mcol+t])
                    Bw = bq.tile([P, NTW, WIN], BF16, tag="Bw")
                    base = meta_bf[:, mcol:mcol + ntw]
                    mb = bass.AP(base.tensor, base.offset,
                                 list(base.ap) + [[0, WIN]])
                    nc.vector.tensor_tensor(
                        out=Bw[:, :ntw, :], in0=iotaB[:, :ntw, :],
                        in1=mb, op=OP.is_equal)
                    return Bw

                def emit_tile(g, slot, Bw, wslot, qpsum, wrel, last):
                    reg = qpsum[:, wrel * WIN:(wrel + 1) * WIN]
                    if node_major:
                        nc.tensor.matmul(out=reg, lhsT=Bw[:, wslot, :],
                                         rhs=g[:, slot, :],
                                         start=False, stop=last)
                    else:
                        nc.tensor.matmul(out=reg, lhsT=g[:, slot, :],
                                         rhs=Bw[:, wslot, :],
                                         start=False, stop=last)

                def table_phase_quad(q, nxt_layer):
                    # hw for quad q of h_out -> T' rows + TpT (for next layer)
                    ncol = min(512, NCP - q * 512)
                    hw_ps = tps.tile([P, 512], F32, tag="hw")
                    nc.tensor.matmul(
                        out=hw_ps[:, :ncol],
                        lhsT=w_bf[:, nxt_layer * H:(nxt_layer + 1) * H],
                        rhs=h_out[:, q * 512:q * 512 + ncol],
                        start=True, stop=True)
                    nc.vector.tensor_tensor(
                        out=TpT[:, q * 512:q * 512 + ncol],
                        in0=hw_ps[:, :ncol],
                        in1=dinvR[:, q * 512:q * 512 + ncol], op=OP.mult)
                    for b in range(4):
                        t = q * 4 + b
                        if t >= W:
                            break
                        nq = min(P, NC - t * P)
                        if nq <= 0:
                            break
                        tp_ps = tps2.tile([P, P], BF16, tag="tr")
                        nc.tensor.transpose(
                            out=tp_ps[:], in_=TpT[:, t * P:(t + 1) * P],
                            identity=ident_bf[:])
                        stg = tbl.tile([P, P], BF16, tag="stg")
                        nc.scalar.activation(out=stg[:], in_=tp_ps[:],
                                             func=AF.Copy)
                        nc.sync.dma_start(
                            out=tab_shard[t * P:t * P + nq, :],
                            in_=stg[:nq, :])

                def pool_quad(q, pool_ps0, pool_ps1, pm, h_src):
                    for b in range(4):
                        t = q * 4 + b
                        if t >= W:
                            break
                        Bp = fl.tile([P, GS], BF16, tag="Bp")
                        nc.vector.tensor_scalar(
                            out=Bp[:], in0=iota_pool[:],
                            scalar1=pm[:, t:t + 1], scalar2=None,
                            op0=OP.is_equal)
                        blk = h_src[:, t * P:(t + 1) * P]
                        nc.tensor.matmul(out=pool_ps0[:], lhsT=Bp[:, :P],
                                         rhs=blk, start=(t == 0),
                                         stop=(t == W - 1))
                        nc.tensor.matmul(out=pool_ps1[:], lhsT=Bp[:, P:],
                                         rhs=blk, start=(t == 0),
                                         stop=(t == W - 1))

                mctr = [0]  # meta column counter (window-major, per layer)
                issue_range(0)
                for q in range(Q):
                    if q + 1 < Q:
                        issue_range(q + 1)
                    ncol = min(512, NCP - q * 512)
                    qpsum = eps.tile([P, 512], F32, tag="qp")
                    # self-loop injection (opens the accumulation group)
                    if node_major:
                        for b in range(4):
                            w = q * 4 + b
                            if w >= W:
                                break
                            nc.tensor.matmul(
                                out=qpsum[:, b * WIN:(b + 1) * WIN],
                                lhsT=TpT[:, w * WIN:(w + 1) * WIN],
                                rhs=ident_bf[:], start=(b == 0), stop=False)
                    else:
                        nc.tensor.matmul(
                            out=qpsum[:, :ncol], lhsT=ident_bf[:],
                            rhs=TpT[:, q * 512:q * 512 + ncol],
                            start=True, stop=False)
                    # edge matmuls
                    nq_tiles = sum(t_cw[c][w] for c in range(CH)
                                   for w in range(q * 4, min(q * 4 + 4, W)))
                    done = 0
                    for b in range(4):
                        w = q * 4 + b
                        if w >= W:
                            break
                        ntw = sum(t_cw[c][w] for c in range(CH))
                        Bw = build_B(mctr[0], ntw)
                        wslot = 0
                        for c in range(CH):
                            g = gbuf[(q, c)]
                            base = gtile[c][q * 4]  # first tile of call
                            for i in range(t_cw[c][w]):
                                slot = (gtile[c][w] - base) + i
                                done += 1
                                emit_tile(g, slot, Bw, wslot, qpsum, b,
                                          done == nq_tiles)
                                wslot += 1
                        mctr[0] += ntw
                    # flush
                    if node_major:
                        for b in range(4):
                            w = q * 4 + b
                            if w >= W:
                                break
                            nc.vector.scalar_tensor_tensor(
                                out=h_out[:, w * WIN:(w + 1) * WIN],
                                in0=qpsum[:, b * WIN:(b + 1) * WIN],
                                scalar=dinv_pm[:, w:w + 1],
                                in1=bias3R[:], op0=OP.mult, op1=OP.add)
                    else:
                        tmp = fl.tile([P, 512], BF16, tag="tmp")
                        nc.vector.tensor_tensor(
                            out=tmp[:, :ncol], in0=qpsum[:, :ncol],
                            in1=dinvR[:, q * 512:q * 512 + ncol], op=OP.mult)
                        nc.scalar.activation(
                            out=h_out[:, q * 512:q * 512 + ncol],
                            in_=tmp[:, :ncol], func=AF.Relu,
                            bias=b_cols[:, layer:layer + 1], scale=1.0)
                    # interleaved next-phase work
                    if layer < cfg.L - 1 and DEBUG_STAGE == 0:
                        table_phase_quad(q, layer + 1)
                if layer < cfg.L - 1:
                    if DEBUG_STAGE != 0:
                        for q in range(Q):
                            table_phase_quad(q, layer + 1)
                    nc.gpsimd.collective_compute(
                        "AllGather", OP.bypass, replica_groups=[cores],
                        ins=[tab_shard[:, :]], outs=[tab_full[:, :]])

        def dump_h(src_tile):
            dbg_d = nc.declare_dram_parameter("dbg", [P, NCP], F32,
                                              isOutput=True)
            with tc.tile_pool(name="dbg", bufs=2) as dbp:
                CWD = 512
                for s0 in range(0, NCP, CWD):
                    nn = min(CWD, NCP - s0)
                    dt_ = dbp.tile([P, CWD], F32, tag="d")
                    nc.vector.tensor_copy(out=dt_[:, :nn],
                                          in_=src_tile[:, s0:s0 + nn])
                    nc.sync.dma_start(out=dbg_d[:, s0:s0 + nn],
                                      in_=dt_[:, :nn])

        edge_pass(0, hT_a, node_major=False)
        if DEBUG_STAGE == 1:
            dump_h(hT_a)
        edge_pass(1, hT_b, node_major=False)
        if DEBUG_STAGE == 2:
            dump_h(hT_b)
        edge_pass(2, h3nm, node_major=True)
        if DEBUG_STAGE == 3:
            dump_h(h3nm)

        # ---------------- pooling ----------------
        with nc.named_scope("pool"), \
             tc.tile_pool(name="po", bufs=3) as po, \
             tc.tile_pool(name="po_ps", bufs=2, space="PSUM") as po_ps, \
             tc.tile_pool(name="po_acc", bufs=2, space="PSUM") as po_acc:
            pm = po.tile([P, W], F32, tag="pm")
            nc.sync.dma_start(out=pm[:], in_=pool_meta[:, :])
            gcols = po.tile([P, 2], I32, tag="gcols")
            nc.sync.dma_start(out=gcols[:], in_=gid_cols[:, :])
            recip_sb = po.tile([P, GW], F32, tag="recip")
            nc.sync.dma_start(out=recip_sb[:], in_=recip_pm[:, :])

            acc0 = po_acc.tile([P, P], F32)
            acc1 = po_acc.tile([P, P], F32)
            for t in range(W):
                Bp = po.tile([P, GS], BF16, tag="Bp")
                nc.vector.tensor_scalar(
                    out=Bp[:], in0=iota_pool[:],
                    scalar1=pm[:, t:t + 1], scalar2=None,
                    op0=OP.is_equal)
                blk = h3nm[:, t * P:(t + 1) * P]
                nc.tensor.matmul(out=acc0[:], lhsT=Bp[:, :P], rhs=blk,
                                 start=(t == 0), stop=(t == W - 1))
                nc.tensor.matmul(out=acc1[:], lhsT=Bp[:, P:], rhs=blk,
                                 start=(t == 0), stop=(t == W - 1))

            def dummy_gather():
                dz = po.tile([P, 1, P], BF16, tag="dz")
                zi = po.tile([P, 8], I16, tag="zi")
                nc.vector.memset(zi[:], 0)
                gi = nc.gpsimd.dma_gather(
                    out_ap=dz[:], in_ap=t1_dram[:, :], idxs_ap=zi[:],
                    num_idxs=P, num_idxs_reg=P, elem_size=H,
                    single_packet=False, queue_num=pd["i"] % NQ)
                chain_pool_dma(gi)

            zt = po.tile([P, P], F32, tag="zt")
            nc.vector.memset(zt[:], 0.0)
            for r0 in range(0, cfg.G + GS, P):
                nc.sync.dma_start(out=pooled_nm[r0:r0 + P, :], in_=zt[:])

            for half, acc in ((0, acc0), (1, acc1)):
                rows = po.tile([P, P], F32, tag="rows")
                nc.scalar.activation(out=rows[:], in_=acc[:], func=AF.Copy)
                while pd["i"] % NQ != 0:
                    dummy_gather()  # scatters run on queue 0: align lane
                si = nc.gpsimd.indirect_dma_start(
                    out=pooled_nm[:, :],
                    out_offset=IndirectOffsetOnAxis(
                        ap=gcols[:, half:half + 1], axis=0),
                    in_=rows[:], in_offset=None)
                chain_pool_dma(si)

            nc.gpsimd.collective_compute(
                "AllReduce", OP.add, replica_groups=[cores],
                ins=[pooled_nm[:, :]], outs=[pooled_sum[:, :]])

            for gw in range(GW):
                ot = po.tile([P, H], F32, tag="ot")
                nc.sync.dma_start(out=ot[:],
                                  in_=pooled_sum[gw * P:(gw + 1) * P, :])
                os = po.tile([P, H], F32, tag="os")
                nc.vector.tensor_scalar(
                    out=os[:], in0=ot[:], scalar1=recip_sb[:, gw:gw + 1],
                    scalar2=None, op0=OP.mult)
                nc.sync.dma_start(out=out_d[gw * P:(gw + 1) * P, :],
                                  in_=os[:])

    return nc


# --------------------------------------------------------------------------
# entry point: full inputs -> full output
# --------------------------------------------------------------------------

_CACHE = {}


def _get_compiled(cfg, st_key, st):
    if st_key not in _CACHE:
        nc = build_nc(cfg, st)
        nc.finalize()
        _CACHE[st_key] = nc
    return _CACHE[st_key]


def kernel(x, edge_index, batch, emb_table, Ws, bs):
    cfg = Cfg()  # full problem size, hardcoded
    st, in_maps = preprocess(x, edge_index, batch, emb_table, Ws, bs, cfg)
    st_key = tuple(tuple(r) for r in st.t_cw)
    nc = _get_compiled(cfg, st_key, st)

    from concourse.bass_utils import run_bass_kernel_spmd

    res = run_bass_kernel_spmd(nc, in_maps, list(range(cfg.C)))
    return np.ascontiguousarray(res.results[0]["out"])


# revision 18
# speedup vs baseline: 1.7067x; 1.0098x over previous
"""GCN embedder kernel for TRN2, 8-core SPMD (v6: 128-wide dst windows,
pure-one-hot scatter matmuls, dinv factoring, unified 3-layer edge pass).

Design
------
* Nodes sharded contiguously across C=8 cores (NC=12500 each). Edges are
  owned by the dst core. Self-loops are NOT materialized as edges.
* Normalization factored: norm(s,d) = dinv[s]*dinv[d]. dinv[src] is
  pre-multiplied into the gather table rows (T'[v] = dinv[v] * (h@W)[v]),
  dinv[dst] is applied at PSUM flush. The self-loop term
  dinv[d]^2*(h@W)[d] = dinv[d]*T'[d] is injected by an identity matmul
  of the local feature-major T' into PSUM before the edge matmuls.
* Edge pass (identical structure for all 3 layers): edges sorted by
  (src_chunk, window, dst) where window = 128 dst nodes. Per-edge-tile
  (128 edges) the one-hot B[e, drel] = is_equal(iota, drel) is ONE DVE
  tensor_scalar (or 2 scalar-engine ACTs for a fraction of tiles); one
  PE matmul accumulates into the 512-wide quad PSUM bank at the window's
  128-col offset. Layers 1-2: lhsT=messages, rhs=B -> feature-major PSUM.
  Layer 3: lhsT=B, rhs=messages -> node-major PSUM (pooling needs
  node-major and layer 3 feeds nothing else).
* Gathers: per (chunk, quad-range) - ~100 calls/layer of ~2-3K indices
  (994ns fixed GpSimd cost per SWDGE call makes small calls ruinous).
  Table rows are bf16 [N,128]=256B. Indices int16 (chunk-relative).
  The SAME index/meta streams serve all 3 layers (layer 1's table is
  per-node: T1n[v] = dinv[v]*T1[x[v]], vocab-gathered on chip).
* Table phase (interleaved into the edge pass of the previous layer at
  quad granularity): hwT = W^T @ hT (PE), T'T = hwT * dinvR (DVE),
  transpose blocks to rows (PE), DMA to DRAM shard; AllGather.
* Pooling: layer-3 node-major blocks feed is_equal(batchrel) one-hot
  matmuls directly (lhsT=Bpool half, rhs=h3 block) accumulating
  [grel<=256, H] in 2 PSUM banks; flush rows scatter by graph id
  (indirect DMA), AllReduce, scale by 1/cnt.

All structure (tile counts, call sizes) is maxed across cores so the
single SPMD program fits every core; pad slots have drel=-1 (zero
one-hot column) and index 0 (valid row).
"""

import math
from contextlib import ExitStack
from dataclasses import dataclass, field

import numpy as np

import concourse.mybir as mybir
import concourse.tile as tile
from concourse import bacc, bass
from concourse.bass import AP, IndirectOffsetOnAxis, ds
from concourse.masks import make_identity

F32 = mybir.dt.float32
BF16 = mybir.dt.bfloat16
I16 = mybir.dt.int16
I32 = mybir.dt.int32
AF = mybir.ActivationFunctionType
OP = mybir.AluOpType

P = 128  # partitions / hidden size / vocab

DEBUG_STAGE = 0  # 0=off; 1..3 = dump h after that layer


@dataclass
class Cfg:
    N: int = 100000
    E: int = 1600000
    H: int = 128
    V: int = 128
    L: int = 3
    G: int = 1024
    C: int = 8          # cores
    CH: int = 4         # gather-table chunks (int16 index limit)
    WIN: int = 128      # dst window width
    NQ: int = 4         # SWDGE queues
    SCALAR_FRAC_NUM: int = 1   # of every DEN tiles, NUM one-hots on Scalar
    SCALAR_FRAC_DEN: int = 7

    @property
    def NC(self):
        assert self.N % self.C == 0
        return self.N // self.C

    @property
    def CHN(self):
        assert self.N % self.CH == 0
        return self.N // self.CH

    @property
    def W(self):  # 128-wide dst windows per core
        return math.ceil(self.NC / self.WIN)

    @property
    def Q(self):  # 512-wide quads (ranges) per core
        return math.ceil(self.W / 4)

    @property
    def NCP(self):
        return self.W * self.WIN

    @property
    def GSPAN(self):
        return 256


@dataclass
class Structure:
    # t_cw[c][w]: edge tiles for (chunk c, window w), maxed across cores
    t_cw: list = field(default_factory=list)

    @property
    def T(self):
        return sum(sum(r) for r in self.t_cw)


def preprocess(x, edge_index, batch, emb_table, Ws, bs, cfg: Cfg):
    """Host-side (index-only) preprocessing."""
    N, E, C, CH = cfg.N, cfg.E, cfg.C, cfg.CH
    NC, CHN, W, Q, WIN = cfg.NC, cfg.CHN, cfg.W, cfg.Q, cfg.WIN

    x = np.asarray(x).astype(np.int64)
    edge_index = np.asarray(edge_index).astype(np.int64)
    batch = np.asarray(batch).astype(np.int64)

    src, dst = edge_index[0], edge_index[1]
    deg = (np.bincount(dst, minlength=N) + 1).astype(np.float32)  # + self
    dinv = (1.0 / np.sqrt(deg)).astype(np.float32)

    owner = dst // NC
    per_core = []
    for c in range(C):
        m = owner == c
        s_c = src[m]
        d_c = dst[m] - c * NC
        w_c = d_c // WIN
        ck = s_c // CHN
        o = np.lexsort((d_c, w_c, ck))  # chunk-major, then window, then dst
        per_core.append(dict(s=s_c[o] % CHN, drel=(d_c[o] % WIN),
                             grp=(ck[o] * W + w_c[o])))

    # tiles per (chunk, window), maxed across cores
    t_cw = np.zeros((CH, W), dtype=np.int64)
    for c in range(C):
        cnt = np.bincount(per_core[c]["grp"], minlength=CH * W)
        t_cw = np.maximum(t_cw, -(-cnt.reshape(CH, W) // P))
    assert (t_cw >= 1).all()
    st = Structure(t_cw=[list(map(int, r)) for r in t_cw])
    T = st.T

    # global tile offset of group (c, w) in the (chunk-major) stream
    gtile = np.zeros((CH, W), dtype=np.int64)
    off = 0
    for c in range(CH):
        for w in range(W):
            gtile[c, w] = off
            off += t_cw[c, w]
    assert off == T

    # gather calls: one per (chunk, quad-range of 4 windows)
    # call (c, q) covers tiles gtile[c, 4q] .. (+ nt(c,q))
    nt_cq = np.zeros((CH, Q), dtype=np.int64)
    for c in range(CH):
        for q in range(Q):
            nt_cq[c, q] = t_cw[c, 4 * q:4 * q + 4].sum()
    NTC = int(nt_cq.max())

    def wrap(arr):
        # [n] int16 -> [128, n//16] wrapped in 16 partitions, tiled x8
        wr = arr.reshape(-1, 16).T
        return np.tile(wr, (8, 1))

    in_maps = []
    for c in range(C):
        pc = per_core[c]
        cnt = np.bincount(pc["grp"], minlength=CH * W).reshape(CH, W)
        starts = np.zeros((CH, W), dtype=np.int64)
        pos = 0
        for cc in range(CH):
            for w in range(W):
                starts[cc, w] = pos
                pos += cnt[cc, w]

        # meta in WINDOW-major (consumption) order: (w, c, i); idx stream in
        # chunk-major (gather) order: (c, w, i)
        meta = np.full((P, T), -1.0, dtype=np.float32)
        idxs = np.zeros(T * P, dtype=np.int16)
        wt0 = 0
        for w in range(W):
            for cc in range(CH):
                n = int(cnt[cc, w])
                sl = slice(int(starts[cc, w]), int(starts[cc, w]) + n)
                ii = np.arange(n)
                meta[ii % P, wt0 + ii // P] = pc["drel"][sl]
                idxs[int(gtile[cc, w]) * P + ii] = pc["s"][sl].astype(np.int16)
                wt0 += int(t_cw[cc, w])
        assert wt0 == T

        # gidx in range-major call order: [q][c] concatenated
        gidx = np.zeros((P, T * 8), dtype=np.int16)
        col = 0
        for q in range(Q):
            for cc in range(CH):
                t0, nt = int(gtile[cc, 4 * q]), int(nt_cq[cc, q])
                gidx[:, col:col + nt * 8] = wrap(idxs[t0 * P:(t0 + nt) * P])
                col += nt * 8
        assert col == T * 8

        # vocab gather indices for the layer-1 table (node-major, pad->0)
        nodes = np.arange(cfg.NCP) + c * NC
        valid = nodes < (c + 1) * NC
        xl = np.where(valid, x[np.minimum(nodes, N - 1)], 0)
        xidx = wrap(xl.astype(np.int16))  # [128, NCP//16*8] = [128, W*64]

        dloc = np.where(valid, dinv[np.minimum(nodes, N - 1)], 0.0)
        dinv_pm = dloc.reshape(W, P).T.copy().astype(np.float32)  # [128, W]
        dinvR = np.broadcast_to(dloc[None, :], (P, cfg.NCP)).astype(np.float32)

        bias3R = np.broadcast_to(np.asarray(bs)[2][None, :],
                                 (P, cfg.H)).astype(np.float32).copy()

        # pooling metadata (as baseline)
        bvals = np.where(valid, batch[np.minimum(nodes, N - 1)], -1)
        gmin = int(batch[c * NC])
        gmax = int(batch[min((c + 1) * NC, N) - 1])
        assert gmax - gmin < cfg.GSPAN, (c, gmin, gmax)
        brel = np.where(valid, bvals - gmin, -1).astype(np.float32)
        pool_meta = brel.reshape(W, P).T.copy()  # [128, W]
        gid_rows = gmin + np.arange(cfg.GSPAN)
        gid_rows = np.where(gid_rows < cfg.G, gid_rows,
                            cfg.G + np.arange(cfg.GSPAN) % 256).astype(np.int32)
        gid_cols = gid_rows.reshape(2, P).T.copy()  # [128, 2]

        cnts = np.bincount(batch, minlength=cfg.G).astype(np.float32)
        recip = 1.0 / np.maximum(cnts, 1.0)
        recip_pm = recip.reshape(cfg.G // P, P).T.copy()

        in_maps.append({
            "meta": meta, "gidx": gidx, "xidx": np.ascontiguousarray(xidx),
            "dinv_pm": dinv_pm, "dinvR": np.ascontiguousarray(dinvR),
            "bias3R": bias3R,
            "pool_meta": pool_meta, "gid_cols": gid_cols,
            "recip_pm": recip_pm,
            "emb": np.asarray(emb_table, dtype=np.float32),
            "Ws": np.asarray(Ws, dtype=np.float32),
            "bs": np.asarray(bs, dtype=np.float32),
        })

    st.nt_cq = [list(map(int, r)) for r in nt_cq]
    st.gtile = [list(map(int, r)) for r in gtile]
    st.NTC = NTC
    st.NTW = int(t_cw.sum(axis=0).max())  # max tiles per window (all chunks)
    return st, in_maps


# --------------------------------------------------------------------------
# device program
# --------------------------------------------------------------------------

def build_nc(cfg: Cfg, st: Structure):
    N, H, C, CH, W, Q = cfg.N, cfg.H, cfg.C, cfg.CH, cfg.W, cfg.Q
    NC, CHN, NCP, WIN = cfg.NC, cfg.CHN, cfg.NCP, cfg.WIN
    T = st.T
    NTC = st.NTC
    GS = cfg.GSPAN
    GW = cfg.G // P
    NQ = cfg.NQ

    nc = bacc.Bacc(None, num_devices=C, num_swdge_queues=NQ)
    cores = list(range(C))

    # ---- external I/O ----
    meta_d = nc.declare_dram_parameter("meta", [P, T], F32, isOutput=False)
    gidx_d = nc.declare_dram_parameter("gidx", [P, T * 8], I16, isOutput=False)
    xidx_d = nc.declare_dram_parameter("xidx", [P, W * 8], I16, isOutput=False)
    dinv_pm_d = nc.declare_dram_parameter("dinv_pm", [P, W], F32, isOutput=False)
    dinvR_d = nc.declare_dram_parameter("dinvR", [P, NCP], F32, isOutput=False)
    bias3R_d = nc.declare_dram_parameter("bias3R", [P, H], F32, isOutput=False)
    pool_meta = nc.declare_dram_parameter("pool_meta", [P, W], F32, isOutput=False)
    gid_cols = nc.declare_dram_parameter("gid_cols", [P, 2], I32, isOutput=False)
    recip_pm = nc.declare_dram_parameter("recip_pm", [P, GW], F32, isOutput=False)
    emb_d = nc.declare_dram_parameter("emb", [P, H], F32, isOutput=False)
    Ws_d = nc.declare_dram_parameter("Ws", [cfg.L, H, H], F32, isOutput=False)
    bs_d = nc.declare_dram_parameter("bs", [cfg.L, H], F32, isOutput=False)
    out_d = nc.declare_dram_parameter("out", [cfg.G, H], F32, isOutput=True)

    # ---- internal DRAM ----
    t1_dram = nc.dram_tensor("t1_tab", [cfg.V, H], BF16)
    tab_shard = nc.dram_tensor("tab_shard", [NC, H], BF16)
    tab_full = nc.dram_tensor("tab_full", [N, H], BF16, addr_space="Shared")
    pooled_nm = nc.dram_tensor("pooled_nm", [cfg.G + GS, H], F32)
    pooled_sum = nc.dram_tensor("pooled_sum", [cfg.G + GS, H], F32,
                                addr_space="Shared")

    from concourse.tile import add_dep_helper
    pd = {"i": 0, "last": None}

    def chain_pool_dma(inst, chain=True):
        if chain and pd["last"] is not None:
            add_dep_helper(inst.ins, pd["last"].ins, sync=False,
                           reason="pool-dma queue/lane parity order")
        pd["last"] = inst
        pd["i"] += 1

    with tile.TileContext(nc) as tc, ExitStack() as ctx:
        const = ctx.enter_context(tc.tile_pool(name="const", bufs=1))
        hpool = ctx.enter_context(tc.tile_pool(name="hbuf", bufs=1))

        ident = const.tile([P, P], F32)
        make_identity(nc, ident[:])
        ident_bf = const.tile([P, P], BF16)
        make_identity(nc, ident_bf[:])
        iota_i = const.tile([P, 512], I32)
        nc.gpsimd.iota(iota_i[:], pattern=[[1, 512]], base=0,
                       channel_multiplier=0)
        iota_pool = const.tile([P, GS], BF16)
        nc.vector.tensor_copy(out=iota_pool[:], in_=iota_i[:, :GS])
        # repeating 0..WIN-1 pattern, one block per tile of a window group
        NTW = st.NTW
        iotaB_i = const.tile([P, NTW, WIN], I32)
        nc.gpsimd.iota(iotaB_i[:], pattern=[[0, NTW], [1, WIN]], base=0,
                       channel_multiplier=0)
        iotaB = const.tile([P, NTW, WIN], BF16)
        nc.vector.tensor_copy(out=iotaB[:], in_=iotaB_i[:])

        b_cols = const.tile([P, cfg.L], F32)
        for l in range(cfg.L):
            nc.sync.dma_start(out=b_cols[:, l:l + 1], in_=bs_d[l, :, None])
        w_bf = const.tile([P, cfg.L * H], BF16, tag="w_bf")
        with tc.tile_pool(name="wload", bufs=2) as wl:
            for l in range(cfg.L):
                wt = wl.tile([P, H], F32, tag="wt")
                nc.sync.dma_start(out=wt[:], in_=Ws_d[l])
                nc.vector.tensor_copy(out=w_bf[:, l * H:(l + 1) * H], in_=wt[:])
        bias3R = const.tile([P, H], F32)
        nc.sync.dma_start(out=bias3R[:], in_=bias3R_d[:, :])
        dinv_pm = const.tile([P, W], F32)
        nc.sync.dma_start(out=dinv_pm[:], in_=dinv_pm_d[:, :])

        # resident meta (bf16: drel in 0..127 and -1 are exact) + dinvR (bf16)
        meta_bf = const.tile([P, T], BF16, tag="meta_bf")
        with tc.tile_pool(name="mld", bufs=2) as mld:
            MC = 1024
            for s0 in range(0, T, MC):
                nn = min(MC, T - s0)
                mt_ = mld.tile([P, MC], F32, tag="m")
                nc.sync.dma_start(out=mt_[:, :nn], in_=meta_d[:, s0:s0 + nn])
                nc.vector.tensor_copy(out=meta_bf[:, s0:s0 + nn],
                                      in_=mt_[:, :nn])
        dinvR = const.tile([P, NCP], BF16, tag="dinvR")
        with tc.tile_pool(name="dld", bufs=2) as dld:
            for s0 in range(0, NCP, 512):
                nn = min(512, NCP - s0)
                dt_ = dld.tile([P, 512], F32, tag="d")
                nc.sync.dma_start(out=dt_[:, :nn], in_=dinvR_d[:, s0:s0 + nn])
                nc.vector.tensor_copy(out=dinvR[:, s0:s0 + nn],
                                      in_=dt_[:, :nn])

        hT_a = hpool.tile([P, NCP], BF16)     # feature-major h (layers 1,2)
        hT_b = hpool.tile([P, NCP], BF16)
        TpT = hpool.tile([P, NCP], BF16)      # feature-major local T'
        h3nm = hT_a                           # layer-3 out (node-major) aliases
        #                                       layer-1 h (dead by then)

        # ---------------- layer-1 table: T1n[v] = dinv[v]*T1[x[v]] ----------
        with nc.named_scope("boot"), \
             tc.tile_pool(name="pro", bufs=2) as pro, \
             tc.tile_pool(name="pro_ps", bufs=2, space="PSUM") as pro_ps, \
             tc.tile_pool(name="bootg", bufs=2) as bootg, \
             tc.tile_pool(name="bootix", bufs=1) as bootix:
            emb_sb = pro.tile([P, H], F32, tag="emb")
            nc.sync.dma_start(out=emb_sb[:], in_=emb_d[:, :])
            w1_sb = pro.tile([P, H], F32, tag="w1")
            nc.sync.dma_start(out=w1_sb[:], in_=Ws_d[0])
            embT_ps = pro_ps.tile([P, P], F32)
            nc.tensor.transpose(out=embT_ps[:], in_=emb_sb[:], identity=ident[:])
            embT = pro.tile([P, P], F32, tag="embT")
            nc.vector.tensor_copy(out=embT[:], in_=embT_ps[:])
            t1t_ps = pro_ps.tile([P, P], F32)
            nc.tensor.matmul(out=t1t_ps[:], lhsT=w1_sb[:], rhs=embT[:],
                             start=True, stop=True)
            t1t = pro.tile([P, P], F32, tag="t1t")
            nc.vector.tensor_copy(out=t1t[:], in_=t1t_ps[:])
            t1nm_ps = pro_ps.tile([P, P], F32)
            nc.tensor.transpose(out=t1nm_ps[:], in_=t1t[:], identity=ident[:])
            t1nm = pro.tile([P, P], BF16, tag="t1nm")
            nc.vector.tensor_copy(out=t1nm[:], in_=t1nm_ps[:])
            nc.sync.dma_start(out=t1_dram[:, :], in_=t1nm[:])

            # vocab gather (node-major), scale by dinv, rows -> tab_shard,
            # transpose -> TpT
            xix = bootix.tile([P, W * 8], I16, tag="xix")
            nc.sync.dma_start(out=xix[:], in_=xidx_d[:, :])
            BG = 14  # tiles per vocab-gather call
            for t0 in range(0, W, BG):
                nt = min(BG, W - t0)
                g = bootg.tile([P, BG, H], BF16, tag="vg")
                nsub = min(NQ, nt)
                per = -(-nt // nsub)
                s0 = 0
                while s0 < nt:
                    sn = min(per, nt - s0)
                    gi = nc.gpsimd.dma_gather(
                        out_ap=g[:, s0:s0 + sn, :], in_ap=t1_dram[:, :],
                        idxs_ap=xix[:, (t0 + s0) * 8:(t0 + s0 + sn) * 8],
                        num_idxs=sn * P, num_idxs_reg=sn * P,
                        elem_size=H, single_packet=False,
                        queue_num=pd["i"] % NQ)
                    chain_pool_dma(gi)
                    s0 += sn
                # scale whole call's tiles by per-node dinv in one DVE op
                mnb = bootg.tile([P, BG, H], BF16, tag="mn")
                dbase = dinv_pm[:, t0:t0 + nt]
                dbc = bass.AP(dbase.tensor, dbase.offset,
                              list(dbase.ap) + [[0, H]])
                nc.vector.tensor_tensor(out=mnb[:, :nt, :], in0=g[:, :nt, :],
                                        in1=dbc, op=OP.mult)
                for i in range(nt):
                    t = t0 + i
                    nq = min(P, NC - t * P)
                    if nq <= 0:
                        break
                    nc.sync.dma_start(out=tab_shard[t * P:t * P + nq, :],
                                      in_=mnb[:nq, i, :])
                    tp_ps = pro_ps.tile([P, P], BF16, tag="tp")
                    nc.tensor.transpose(out=tp_ps[:], in_=mnb[:, i, :],
                                        identity=ident_bf[:])
                    nc.scalar.activation(out=TpT[:, t * P:(t + 1) * P],
                                         in_=tp_ps[:], func=AF.Copy)
            nc.gpsimd.collective_compute(
                "AllGather", OP.bypass, replica_groups=[cores],
                ins=[tab_shard[:, :]], outs=[tab_full[:, :]])

        # ---------------- unified edge pass ----------------
        tilectr = [0]
        t_cw = st.t_cw
        nt_cq = st.nt_cq
        gtile = st.gtile

        # gidx column offset of call (q, c) in range-major layout
        gcol = {}
        col = 0
        for q in range(Q):
            for c in range(CH):
                gcol[(q, c)] = col
                col += nt_cq[c][q] * 8
        qcol = {}  # column offset of range q's gidx block, and width
        for q in range(Q):
            qcol[q] = (gcol[(q, 0)],
                       sum(nt_cq[c][q] for c in range(CH)) * 8)

        def edge_pass(layer, h_out, node_major):
            lname = f"layer{layer + 1}"
            with nc.named_scope(lname), \
                 tc.tile_pool(name=f"ix{layer}", bufs=2) as ixp, \
                 tc.tile_pool(name=f"gb{layer}", bufs=2) as gb, \
                 tc.tile_pool(name=f"bq{layer}", bufs=3) as bq, \
                 tc.tile_pool(name=f"fl{layer}", bufs=3) as fl, \
                 tc.tile_pool(name=f"eps{layer}", bufs=3, space="PSUM") as eps, \
                 tc.tile_pool(name=f"tps{layer}", bufs=2, space="PSUM") as tps, \
                 tc.tile_pool(name=f"tps2{layer}", bufs=2, space="PSUM") as tps2, \
                 tc.tile_pool(name=f"tbl{layer}", bufs=3) as tbl:

                gbuf = {}

                def issue_range(q):
                    c0, cw = qcol[q]
                    gx = ixp.tile([P, max(w for _, w in qcol.values())],
                                  I16, tag="gx")
                    nc.sync.dma_start(out=gx[:, :cw],
                                      in_=gidx_d[:, c0:c0 + cw])
                    for c in range(CH):
                        nt = nt_cq[c][q]
                        g = gb.tile([P, NTC, H], BF16, tag=f"g{c}")
                        goff = gcol[(q, c)] - c0
                        # split across all queues; batched sub-calls let the
                        # SWDGE ucode generate descriptors on all 4 queue
                        # engines concurrently (~4x effective rate)
                        nsub = min(NQ, nt)
                        per = -(-nt // nsub)
                        s0 = 0
                        while s0 < nt:
                            sn = min(per, nt - s0)
                            gi = nc.gpsimd.dma_gather(
                                out_ap=g[:, s0:s0 + sn, :],
                                in_ap=tab_full[c * CHN:(c + 1) * CHN, :],
                                idxs_ap=gx[:, goff + s0 * 8:
                                           goff + (s0 + sn) * 8],
                                num_idxs=sn * P, num_idxs_reg=sn * P,
                                elem_size=H, single_packet=False,
                                queue_num=pd["i"] % NQ)
                            chain_pool_dma(gi, chain=False)
                            s0 += sn
                        gbuf[(q, c)] = g

                def build_B(mcol, ntw):
                    # one-hot rows for a whole window group in ONE DVE op:
                    # B[p, t, j] = (iota[j] == drel[p, mcol+t])
                    Bw = bq.tile([P, NTW, WIN], BF16, tag="Bw")
                    base = meta_bf[:, mcol:mcol + ntw]
                    mb = bass.AP(base.tensor, base.offset,
                                 list(base.ap) + [[0, WIN]])
                    nc.vector.tensor_tensor(
                        out=Bw[:, :ntw, :], in0=iotaB[:, :ntw, :],
                        in1=mb, op=OP.is_equal)
                    return Bw

                def emit_tile(g, slot, Bw, wslot, qpsum, wrel, last):
                    reg = qpsum[:, wrel * WIN:(wrel + 1) * WIN]
                    if node_major:
                        nc.tensor.matmul(out=reg, lhsT=Bw[:, wslot, :],
                                         rhs=g[:, slot, :],
                                         start=False, stop=last)
                    else:
                        nc.tensor.matmul(out=reg, lhsT=g[:, slot, :],
                                         rhs=Bw[:, wslot, :],
                                         start=False, stop=last)

                def table_phase_quad(q, nxt_layer):
                    # hw for quad q of h_out -> T' rows + TpT (for next layer)
                    ncol = min(512, NCP - q * 512)
                    hw_ps = tps.tile([P, 512], F32, tag="hw")
                    nc.tensor.matmul(
                        out=hw_ps[:, :ncol],
                        lhsT=w_bf[:, nxt_layer * H:(nxt_layer + 1) * H],
                        rhs=h_out[:, q * 512:q * 512 + ncol],
                        start=True, stop=True)
                    nc.vector.tensor_tensor(
                        out=TpT[:, q * 512:q * 512 + ncol],
                        in0=hw_ps[:, :ncol],
                        in1=dinvR[:, q * 512:q * 512 + ncol], op=OP.mult)
                    for b in range(4):
                        t = q * 4 + b
                        if t >= W:
                            break
                        nq = min(P, NC - t * P)
                        if nq <= 0:
                            break
                        tp_ps = tps2.tile([P, P], BF16, tag="tr")
                        nc.tensor.transpose(
                            out=tp_ps[:], in_=TpT[:, t * P:(t + 1) * P],
                            identity=ident_bf[:])
                        stg = tbl.tile([P, P], BF16, tag="stg")
                        nc.scalar.activation(out=stg[:], in_=tp_ps[:],
                                             func=AF.Copy)
                        nc.sync.dma_start(
                            out=tab_shard[t * P:t * P + nq, :],
                            in_=stg[:nq, :])

                def pool_quad(q, pool_ps0, pool_ps1, pm, h_src):
                    for b in range(4):
                        t = q * 4 + b
                        if t >= W:
                            break
                        Bp = fl.tile([P, GS], BF16, tag="Bp")
                        nc.vector.tensor_scalar(
                            out=Bp[:], in0=iota_pool[:],
                            scalar1=pm[:, t:t + 1], scalar2=None,
                            op0=OP.is_equal)
                        blk = h_src[:, t * P:(t + 1) * P]
                        nc.tensor.matmul(out=pool_ps0[:], lhsT=Bp[:, :P],
                                         rhs=blk, start=(t == 0),
                                         stop=(t == W - 1))
                        nc.tensor.matmul(out=pool_ps1[:], lhsT=Bp[:, P:],
                                         rhs=blk, start=(t == 0),
                                         stop=(t == W - 1))

                mctr = [0]  # meta column counter (window-major, per layer)
                issue_range(0)
                for q in range(Q):
                    if q + 1 < Q:
                        issue_range(q + 1)
                    ncol = min(512, NCP - q * 512)
                    qpsum = eps.tile([P, 512], F32, tag="qp")
                    # self-loop injection (opens the accumulation group)
                    if node_major:
                        for b in range(4):
                            w = q * 4 + b
                            if w >= W:
                                break
                            nc.tensor.matmul(
                                out=qpsum[:, b * WIN:(b + 1) * WIN],
                                lhsT=TpT[:, w * WIN:(w + 1) * WIN],
                                rhs=ident_bf[:], start=(b == 0), stop=False)
                    else:
                        nc.tensor.matmul(
                            out=qpsum[:, :ncol], lhsT=ident_bf[:],
                            rhs=TpT[:, q * 512:q * 512 + ncol],
                            start=True, stop=False)
                    # edge matmuls
                    nq_tiles = sum(t_cw[c][w] for c in range(CH)
                                   for w in range(q * 4, min(q * 4 + 4, W)))
                    done = 0
                    for b in range(4):
                        w = q * 4 + b
                        if w >= W:
                            break
                        ntw = sum(t_cw[c][w] for c in range(CH))
                        Bw = build_B(mctr[0], ntw)
                        wslot = 0
                        for c in range(CH):
                            g = gbuf[(q, c)]
                            base = gtile[c][q * 4]  # first tile of call
                            for i in range(t_cw[c][w]):
                                slot = (gtile[c][w] - base) + i
                                done += 1
                                emit_tile(g, slot, Bw, wslot, qpsum, b,
                                          done == nq_tiles)
                                wslot += 1
                        mctr[0] += ntw
                    # flush
                    if node_major:
                        for b in range(4):
                            w = q * 4 + b
                            if w >= W:
                                break
                            nc.vector.scalar_tensor_tensor(
                                out=h_out[:, w * WIN:(w + 1) * WIN],
                                in0=qpsum[:, b * WIN:(b + 1) * WIN],
                                scalar=dinv_pm[:, w:w + 1],
                                in1=bias3R[:], op0=OP.mult, op1=OP.add)
                    else:
                        tmp = fl.tile([P, 512], BF16, tag="tmp")
                        nc.vector.tensor_tensor(
                            out=tmp[:, :ncol], in0=qpsum[:, :ncol],
                            in1=dinvR[:, q * 512:q * 512 + ncol], op=OP.mult)
                        nc.scalar.activation(
                            out=h_out[:, q * 512:q * 512 + ncol],
                            in_=tmp[:, :ncol], func=AF.Relu,
                            bias=b_cols[:, layer:layer + 1], scale=1.0)
                    # interleaved next-phase work
                    if layer < cfg.L - 1 and DEBUG_STAGE == 0:
                        table_phase_quad(q, layer + 1)
                if layer < cfg.L - 1:
                    if DEBUG_STAGE != 0:
                        for q in range(Q):
                            table_phase_quad(q, layer + 1)
                    nc.gpsimd.collective_compute(
                        "AllGather", OP.bypass, replica_groups=[cores],
                        ins=[tab_shard[:, :]], outs=[tab_full[:, :]])

        def dump_h(src_tile):
            dbg_d = nc.declare_dram_parameter("dbg", [P, NCP], F32,
                                              isOutput=True)
            with tc.tile_pool(name="dbg", bufs=2) as dbp:
                CWD = 512
                for s0 in range(0, NCP, CWD):
                    nn = min(CWD, NCP - s0)
                    dt_ = dbp.tile([P, CWD], F32, tag="d")
                    nc.vector.tensor_copy(out=dt_[:, :nn],
                                          in_=src_tile[:, s0:s0 + nn])
                    nc.sync.dma_start(out=dbg_d[:, s0:s0 + nn],
                                      in_=dt_[:, :nn])

        edge_pass(0, hT_a, node_major=False)
        if DEBUG_STAGE == 1:
            dump_h(hT_a)
        edge_pass(1, hT_b, node_major=False)
        if DEBUG_STAGE == 2:
            dump_h(hT_b)
        edge_pass(2, h3nm, node_major=True)
        if DEBUG_STAGE == 3:
            dump_h(h3nm)

        # ---------------- pooling ----------------
        with nc.named_scope("pool"), \
             tc.tile_pool(name="po", bufs=3) as po, \
             tc.tile_pool(name="po_ps", bufs=2, space="PSUM") as po_ps, \
             tc.tile_pool(name="po_acc", bufs=2, space="PSUM") as po_acc:
            pm = po.tile([P, W], F32, tag="pm")
            nc.sync.dma_start(out=pm[:], in_=pool_meta[:, :])
            gcols = po.tile([P, 2], I32, tag="gcols")
            nc.sync.dma_start(out=gcols[:], in_=gid_cols[:, :])
            recip_sb = po.tile([P, GW], F32, tag="recip")
            nc.sync.dma_start(out=recip_sb[:], in_=recip_pm[:, :])

            acc0 = po_acc.tile([P, P], F32)
            acc1 = po_acc.tile([P, P], F32)
            for t in range(W):
                Bp = po.tile([P, GS], BF16, tag="Bp")
                nc.vector.tensor_scalar(
                    out=Bp[:], in0=iota_pool[:],
                    scalar1=pm[:, t:t + 1], scalar2=None,
                    op0=OP.is_equal)
                blk = h3nm[:, t * P:(t + 1) * P]
                nc.tensor.matmul(out=acc0[:], lhsT=Bp[:, :P], rhs=blk,
                                 start=(t == 0), stop=(t == W - 1))
                nc.tensor.matmul(out=acc1[:], lhsT=Bp[:, P:], rhs=blk,
                                 start=(t == 0), stop=(t == W - 1))

            def dummy_gather():
                dz = po.tile([P, 1, P], BF16, tag="dz")
                zi = po.tile([P, 8], I16, tag="zi")
                nc.vector.memset(zi[:], 0)
                gi = nc.gpsimd.dma_gather(
                    out_ap=dz[:], in_ap=t1_dram[:, :], idxs_ap=zi[:],
                    num_idxs=P, num_idxs_reg=P, elem_size=H,
                    single_packet=False, queue_num=pd["i"] % NQ)
                chain_pool_dma(gi)

            zt = po.tile([P, P], F32, tag="zt")
            nc.vector.memset(zt[:], 0.0)
            for r0 in range(0, cfg.G + GS, P):
                nc.sync.dma_start(out=pooled_nm[r0:r0 + P, :], in_=zt[:])

            for half, acc in ((0, acc0), (1, acc1)):
                rows = po.tile([P, P], F32, tag="rows")
                nc.scalar.activation(out=rows[:], in_=acc[:], func=AF.Copy)
                while pd["i"] % NQ != 0:
                    dummy_gather()  # scatters run on queue 0: align lane
                si = nc.gpsimd.indirect_dma_start(
                    out=pooled_nm[:, :],
                    out_offset=IndirectOffsetOnAxis(
                        ap=gcols[:, half:half + 1], axis=0),
                    in_=rows[:], in_offset=None)
                chain_pool_dma(si)

            nc.gpsimd.collective_compute(
                "AllReduce", OP.add, replica_groups=[cores],
                ins=[pooled_nm[:, :]], outs=[pooled_sum[:, :]])

            for gw in range(GW):
                ot = po.tile([P, H], F32, tag="ot")
                nc.sync.dma_start(out=ot[:],
                                  in_=pooled_sum[gw * P:(gw + 1) * P, :])
                os = po.tile([P, H], F32, tag="os")
                nc.vector.tensor_scalar(
                    out=os[:], in0=ot[:], scalar1=recip_sb[:, gw:gw + 1],
                    scalar2=None, op0=OP.mult)
                nc.sync.dma_start(out=out_d[gw * P:(gw + 1) * P, :],
                                  in_=os[:])

    return nc


# --------------------------------------------------------------------------
# entry point: full inputs -> full output
# --------------------------------------------------------------------------

_CACHE = {}


def _get_compiled(cfg, st_key, st):
    if st_key not in _CACHE:
        nc = build_nc(cfg, st)
        nc.finalize()
        _CACHE[st_key] = nc
    return _CACHE[st_key]


def kernel(x, edge_index, batch, emb_table, Ws, bs):
    cfg = Cfg()  # full problem size, hardcoded
    st, in_maps = preprocess(x, edge_index, batch, emb_table, Ws, bs, cfg)
    st_key = tuple(tuple(r) for r in st.t_cw)
    nc = _get_compiled(cfg, st_key, st)

    from concourse.bass_utils import run_bass_kernel_spmd

    res = run_bass_kernel_spmd(nc, in_maps, list(range(cfg.C)))
    return np.ascontiguousarray(res.results[0]["out"])


# revision 33
# speedup vs baseline: 1.7771x; 1.0412x over previous
"""GCN embedder kernel for TRN2, 8-core SPMD (v6: 128-wide dst windows,
pure-one-hot scatter matmuls, dinv factoring, unified 3-layer edge pass).

Design
------
* Nodes sharded contiguously across C=8 cores (NC=12500 each). Edges are
  owned by the dst core. Self-loops are NOT materialized as edges.
* Normalization factored: norm(s,d) = dinv[s]*dinv[d]. dinv[src] is
  pre-multiplied into the gather table rows (T'[v] = dinv[v] * (h@W)[v]),
  dinv[dst] is applied at PSUM flush. The self-loop term
  dinv[d]^2*(h@W)[d] = dinv[d]*T'[d] is injected by an identity matmul
  of the local feature-major T' into PSUM before the edge matmuls.
* Edge pass (identical structure for all 3 layers): edges sorted by
  (src_chunk, window, dst) where window = 128 dst nodes. Per-edge-tile
  (128 edges) the one-hot B[e, drel] = is_equal(iota, drel) is ONE DVE
  tensor_scalar (or 2 scalar-engine ACTs for a fraction of tiles); one
  PE matmul accumulates into the 512-wide quad PSUM bank at the window's
  128-col offset. Layers 1-2: lhsT=messages, rhs=B -> feature-major PSUM.
  Layer 3: lhsT=B, rhs=messages -> node-major PSUM (pooling needs
  node-major and layer 3 feeds nothing else).
* Gathers: per (chunk, quad-range) - ~100 calls/layer of ~2-3K indices
  (994ns fixed GpSimd cost per SWDGE call makes small calls ruinous).
  Table rows are bf16 [N,128]=256B. Indices int16 (chunk-relative).
  The SAME index/meta streams serve all 3 layers (layer 1's table is
  per-node: T1n[v] = dinv[v]*T1[x[v]], vocab-gathered on chip).
* Table phase (interleaved into the edge pass of the previous layer at
  quad granularity): hwT = W^T @ hT (PE), T'T = hwT * dinvR (DVE),
  transpose blocks to rows (PE), DMA to DRAM shard; AllGather.
* Pooling: layer-3 node-major blocks feed is_equal(batchrel) one-hot
  matmuls directly (lhsT=Bpool half, rhs=h3 block) accumulating
  [grel<=256, H] in 2 PSUM banks; flush rows scatter by graph id
  (indirect DMA), AllReduce, scale by 1/cnt.

All structure (tile counts, call sizes) is maxed across cores so the
single SPMD program fits every core; pad slots have drel=-1 (zero
one-hot column) and index 0 (valid row).
"""

import math
from contextlib import ExitStack
from dataclasses import dataclass, field

import numpy as np

import concourse.mybir as mybir
import concourse.tile as tile
from concourse import bacc, bass
from concourse.bass import AP, IndirectOffsetOnAxis, ds
from concourse.masks import make_identity

F32 = mybir.dt.float32
BF16 = mybir.dt.bfloat16
I16 = mybir.dt.int16
I32 = mybir.dt.int32
AF = mybir.ActivationFunctionType
OP = mybir.AluOpType

P = 128  # partitions / hidden size / vocab

DEBUG_STAGE = 0  # 0=off; 1..3 = dump h after that layer


@dataclass
class Cfg:
    N: int = 100000
    E: int = 1600000
    H: int = 128
    V: int = 128
    L: int = 3
    G: int = 1024
    C: int = 8          # cores
    CH: int = 4         # gather-table chunks (int16 index limit)
    WIN: int = 128      # dst window width
    NQ: int = 4         # SWDGE queues
    SCALAR_FRAC_NUM: int = 1   # of every DEN tiles, NUM one-hots on Scalar
    SCALAR_FRAC_DEN: int = 7

    @property
    def NC(self):
        assert self.N % self.C == 0
        return self.N // self.C

    @property
    def CHN(self):
        assert self.N % self.CH == 0
        return self.N // self.CH

    @property
    def W(self):  # 128-wide dst windows per core
        return math.ceil(self.NC / self.WIN)

    @property
    def Q(self):  # 512-wide quads (ranges) per core
        return math.ceil(self.W / 4)

    @property
    def NCP(self):
        return self.W * self.WIN

    @property
    def GSPAN(self):
        return 256


@dataclass
class Structure:
    # t_cw[c][w]: edge tiles for (chunk c, window w), maxed across cores
    t_cw: list = field(default_factory=list)

    @property
    def T(self):
        return sum(sum(r) for r in self.t_cw)


def preprocess(x, edge_index, batch, emb_table, Ws, bs, cfg: Cfg):
    """Host-side (index-only) preprocessing."""
    N, E, C, CH = cfg.N, cfg.E, cfg.C, cfg.CH
    NC, CHN, W, Q, WIN = cfg.NC, cfg.CHN, cfg.W, cfg.Q, cfg.WIN

    x = np.asarray(x).astype(np.int64)
    edge_index = np.asarray(edge_index).astype(np.int64)
    batch = np.asarray(batch).astype(np.int64)

    src, dst = edge_index[0], edge_index[1]
    deg = (np.bincount(dst, minlength=N) + 1).astype(np.float32)  # + self
    dinv = (1.0 / np.sqrt(deg)).astype(np.float32)

    owner = dst // NC
    per_core = []
    for c in range(C):
        m = owner == c
        s_c = src[m]
        d_c = dst[m] - c * NC
        w_c = d_c // WIN
        ck = s_c // CHN
        o = np.lexsort((d_c, ck, w_c))  # window-major, then chunk, then dst
        per_core.append(dict(s=(s_c[o] % CHN), drel=(d_c[o] % WIN),
                             grp=(w_c[o] * CH + ck[o])))

    # tiles per (window, chunk) group, maxed across cores
    NG = W * CH
    t_g = np.zeros(NG, dtype=np.int64)
    for c in range(C):
        cnt = np.bincount(per_core[c]["grp"], minlength=NG)
        t_g = np.maximum(t_g, -(-cnt // P))
    t_g = np.maximum(t_g, 1)
    st = Structure(t_cw=[list(map(int, t_g))])
    T = int(t_g.sum())
    gt0 = np.concatenate([[0], np.cumsum(t_g)[:-1]])  # tile offset per group

    def wrap(arr):
        # [n] int16 -> [128, n//16] wrapped in 16 partitions, tiled x8
        wr = arr.reshape(-1, 16).T
        return np.tile(wr, (8, 1))

    in_maps = []
    for c in range(C):
        pc = per_core[c]
        cnt = np.bincount(pc["grp"], minlength=NG)
        starts = np.concatenate([[0], np.cumsum(cnt)[:-1]])

        # meta + idx stream, both in (window, chunk) consumption order;
        # pads: drel=-1 (zero one-hot col), idx=-1 (trailing - skipped by
        # the gather via the per-core real-count register)
        meta = np.full((P, T), -1.0, dtype=np.float32)
        idxs = np.zeros(T * P, dtype=np.int16)
        for g in range(NG):
            n = int(cnt[g])
            sl = slice(int(starts[g]), int(starts[g]) + n)
            ii = np.arange(n)
            t0 = int(gt0[g])
            meta[ii % P, t0 + ii // P] = pc["drel"][sl]
            idxs[t0 * P + ii] = pc["s"][sl].astype(np.int16)
        gidx = np.zeros((P, T * 8), dtype=np.int16)
        for g in range(NG):
            t0, nt = int(gt0[g]), int(t_g[g])
            gidx[:, t0 * 8:(t0 + nt) * 8] = wrap(idxs[t0 * P:(t0 + nt) * P])
        cnts = np.broadcast_to(cnt.astype(np.int32)[None, :],
                               (P, NG)).copy()

        # vocab gather indices for the layer-1 table (node-major, pad->0)
        nodes = np.arange(cfg.NCP) + c * NC
        valid = nodes < (c + 1) * NC
        xl = np.where(valid, x[np.minimum(nodes, N - 1)], 0)
        xidx = wrap(xl.astype(np.int16))  # [128, W*8]

        dloc = np.where(valid, dinv[np.minimum(nodes, N - 1)], 0.0)
        dinv_pm = dloc.reshape(W, P).T.copy().astype(np.float32)  # [128, W]
        dinvR = np.broadcast_to(dloc[None, :], (P, cfg.NCP)).astype(np.float32)

        bias3R = np.broadcast_to(np.asarray(bs)[2][None, :],
                                 (P, cfg.H)).astype(np.float32).copy()

        # pooling metadata (as baseline)
        bvals = np.where(valid, batch[np.minimum(nodes, N - 1)], -1)
        gmin = int(batch[c * NC])
        gmax = int(batch[min((c + 1) * NC, N) - 1])
        assert gmax - gmin < cfg.GSPAN, (c, gmin, gmax)
        brel = np.where(valid, bvals - gmin, -1).astype(np.float32)
        pool_meta = brel.reshape(W, P).T.copy()  # [128, W]
        gid_rows = gmin + np.arange(cfg.GSPAN)
        gid_rows = np.where(gid_rows < cfg.G, gid_rows,
                            cfg.G + np.arange(cfg.GSPAN) % 256).astype(np.int32)
        gid_cols = gid_rows.reshape(2, P).T.copy()  # [128, 2]

        cnts = np.bincount(batch, minlength=cfg.G).astype(np.float32)
        recip = 1.0 / np.maximum(cnts, 1.0)
        recip_pm = recip.reshape(cfg.G // P, P).T.copy()

        in_maps.append({
            "meta": meta, "gidx": gidx,
            "xidx": np.ascontiguousarray(xidx),
            "dinv_pm": dinv_pm, "dinvR": np.ascontiguousarray(dinvR),
            "bias3R": bias3R,
            "pool_meta": pool_meta, "gid_cols": gid_cols,
            "recip_pm": recip_pm,
            "emb": np.asarray(emb_table, dtype=np.float32),
            "Ws": np.asarray(Ws, dtype=np.float32),
            "bs": np.asarray(bs, dtype=np.float32),
        })

    # per-window tiles and per-quad totals
    t_w = t_g.reshape(W, CH).sum(axis=1)
    nt_q = [int(t_w[4 * q:min(4 * q + 4, W)].sum()) for q in range(Q)]
    st.t_g = [int(v) for v in t_g]
    st.t_w = [int(v) for v in t_w]
    st.nt_q = nt_q
    st.NTQ = max(nt_q)
    st.NTW = int(t_w.max())
    st.NG = NG
    return st, in_maps


# --------------------------------------------------------------------------
# device program
# --------------------------------------------------------------------------

def build_nc(cfg: Cfg, st: Structure):
    N, H, C, CH, W, Q = cfg.N, cfg.H, cfg.C, cfg.CH, cfg.W, cfg.Q
    NC, CHN, NCP, WIN = cfg.NC, cfg.CHN, cfg.NCP, cfg.WIN
    T = st.T
    NTQ = st.NTQ
    GS = cfg.GSPAN
    GW = cfg.G // P
    NQ = cfg.NQ

    nc = bacc.Bacc(None, num_devices=C, num_swdge_queues=NQ)
    cores = list(range(C))

    # ---- external I/O ----
    meta_d = nc.declare_dram_parameter("meta", [P, T], F32, isOutput=False)
    gidx_d = nc.declare_dram_parameter("gidx", [P, T * 8], I16, isOutput=False)
    xidx_d = nc.declare_dram_parameter("xidx", [P, W * 8], I16, isOutput=False)
    dinv_pm_d = nc.declare_dram_parameter("dinv_pm", [P, W], F32, isOutput=False)
    dinvR_d = nc.declare_dram_parameter("dinvR", [P, NCP], F32, isOutput=False)
    bias3R_d = nc.declare_dram_parameter("bias3R", [P, H], F32, isOutput=False)
    pool_meta = nc.declare_dram_parameter("pool_meta", [P, W], F32, isOutput=False)
    gid_cols = nc.declare_dram_parameter("gid_cols", [P, 2], I32, isOutput=False)
    recip_pm = nc.declare_dram_parameter("recip_pm", [P, GW], F32, isOutput=False)
    emb_d = nc.declare_dram_parameter("emb", [P, H], F32, isOutput=False)
    Ws_d = nc.declare_dram_parameter("Ws", [cfg.L, H, H], F32, isOutput=False)
    bs_d = nc.declare_dram_parameter("bs", [cfg.L, H], F32, isOutput=False)
    out_d = nc.declare_dram_parameter("out", [cfg.G, H], F32, isOutput=True)

    # ---- internal DRAM ----
    t1_dram = nc.dram_tensor("t1_tab", [cfg.V, H], BF16)
    tab_shard = nc.dram_tensor("tab_shard", [NC, H], BF16)
    tab_full = nc.dram_tensor("tab_full", [N, H], BF16, addr_space="Shared")
    pooled_nm = nc.dram_tensor("pooled_nm", [cfg.G + GS, H], F32)
    pooled_sum = nc.dram_tensor("pooled_sum", [cfg.G + GS, H], F32,
                                addr_space="Shared")

    from concourse.tile import add_dep_helper
    pd = {"i": 0, "last": None}

    def chain_pool_dma(inst, chain=True):
        if chain and pd["last"] is not None:
            add_dep_helper(inst.ins, pd["last"].ins, sync=False,
                           reason="pool-dma queue/lane parity order")
        pd["last"] = inst
        pd["i"] += 1

    with tile.TileContext(nc) as tc, ExitStack() as ctx:
        const = ctx.enter_context(tc.tile_pool(name="const", bufs=1))
        hpool = ctx.enter_context(tc.tile_pool(name="hbuf", bufs=1))

        ident = const.tile([P, P], F32)
        make_identity(nc, ident[:])
        ident_bf = const.tile([P, P], BF16)
        make_identity(nc, ident_bf[:])
        iota_i = const.tile([P, 512], I32)
        nc.gpsimd.iota(iota_i[:], pattern=[[1, 512]], base=0,
                       channel_multiplier=0)
        iota_pool = const.tile([P, GS], BF16)
        nc.vector.tensor_copy(out=iota_pool[:], in_=iota_i[:, :GS])
        # repeating 0..WIN-1 pattern, one block per tile of a window group
        NTW = st.NTW
        iotaB_i = const.tile([P, NTW, WIN], I32)
        nc.gpsimd.iota(iotaB_i[:], pattern=[[0, NTW], [1, WIN]], base=0,
                       channel_multiplier=0)
        iotaB = const.tile([P, NTW, WIN], BF16)
        nc.vector.tensor_copy(out=iotaB[:], in_=iotaB_i[:])

        b_cols = const.tile([P, cfg.L], F32)
        for l in range(cfg.L):
            nc.sync.dma_start(out=b_cols[:, l:l + 1], in_=bs_d[l, :, None])
        w_bf = const.tile([P, cfg.L * H], BF16, tag="w_bf")
        with tc.tile_pool(name="wload", bufs=2) as wl:
            for l in range(cfg.L):
                wt = wl.tile([P, H], F32, tag="wt")
                nc.sync.dma_start(out=wt[:], in_=Ws_d[l])
                nc.vector.tensor_copy(out=w_bf[:, l * H:(l + 1) * H], in_=wt[:])
        bias3R = const.tile([P, H], F32)
        nc.sync.dma_start(out=bias3R[:], in_=bias3R_d[:, :])
        dinv_pm = const.tile([P, W], F32)
        nc.sync.dma_start(out=dinv_pm[:], in_=dinv_pm_d[:, :])

        # resident meta (bf16: drel in 0..127 and -1 are exact) + dinvR (bf16)
        meta_bf = const.tile([P, T], BF16, tag="meta_bf")
        with tc.tile_pool(name="mld", bufs=2) as mld:
            MC = 1024
            for s0 in range(0, T, MC):
                nn = min(MC, T - s0)
                mt_ = mld.tile([P, MC], F32, tag="m")
                nc.sync.dma_start(out=mt_[:, :nn], in_=meta_d[:, s0:s0 + nn])
                nc.vector.tensor_copy(out=meta_bf[:, s0:s0 + nn],
                                      in_=mt_[:, :nn])
        dinvR = const.tile([P, NCP], BF16, tag="dinvR")
        with tc.tile_pool(name="dld", bufs=2) as dld:
            for s0 in range(0, NCP, 512):
                nn = min(512, NCP - s0)
                dt_ = dld.tile([P, 512], F32, tag="d")
                nc.sync.dma_start(out=dt_[:, :nn], in_=dinvR_d[:, s0:s0 + nn])
                nc.vector.tensor_copy(out=dinvR[:, s0:s0 + nn],
                                      in_=dt_[:, :nn])

        hT_a = hpool.tile([P, NCP], BF16)     # feature-major h (layers 1,2)
        hT_b = hpool.tile([P, NCP], BF16)
        TpT = hpool.tile([P, NCP], BF16)      # feature-major local T'
        h3nm = hT_a                           # layer-3 out (node-major) aliases
        #                                       layer-1 h (dead by then)

        # ---------------- layer-1 table: T1n[v] = dinv[v]*T1[x[v]] ----------
        with nc.named_scope("boot"), \
             tc.tile_pool(name="pro", bufs=2) as pro, \
             tc.tile_pool(name="pro_ps", bufs=2, space="PSUM") as pro_ps, \
             tc.tile_pool(name="bootg", bufs=2) as bootg, \
             tc.tile_pool(name="bootix", bufs=1) as bootix:
            emb_sb = pro.tile([P, H], F32, tag="emb")
            nc.sync.dma_start(out=emb_sb[:], in_=emb_d[:, :])
            w1_sb = pro.tile([P, H], F32, tag="w1")
            nc.sync.dma_start(out=w1_sb[:], in_=Ws_d[0])
            embT_ps = pro_ps.tile([P, P], F32)
            nc.tensor.transpose(out=embT_ps[:], in_=emb_sb[:], identity=ident[:])
            embT = pro.tile([P, P], F32, tag="embT")
            nc.vector.tensor_copy(out=embT[:], in_=embT_ps[:])
            t1t_ps = pro_ps.tile([P, P], F32)
            nc.tensor.matmul(out=t1t_ps[:], lhsT=w1_sb[:], rhs=embT[:],
                             start=True, stop=True)
            t1t = pro.tile([P, P], F32, tag="t1t")
            nc.vector.tensor_copy(out=t1t[:], in_=t1t_ps[:])
            t1nm_ps = pro_ps.tile([P, P], F32)
            nc.tensor.transpose(out=t1nm_ps[:], in_=t1t[:], identity=ident[:])
            t1nm = pro.tile([P, P], BF16, tag="t1nm")
            nc.vector.tensor_copy(out=t1nm[:], in_=t1nm_ps[:])
            nc.sync.dma_start(out=t1_dram[:, :], in_=t1nm[:])

            # vocab gather (node-major), scale by dinv, rows -> tab_shard,
            # transpose -> TpT
            xix = bootix.tile([P, W * 8], I16, tag="xix")
            nc.sync.dma_start(out=xix[:], in_=xidx_d[:, :])
            BG = 14  # tiles per vocab-gather call
            for t0 in range(0, W, BG):
                nt = min(BG, W - t0)
                g = bootg.tile([P, BG, H], BF16, tag="vg")
                nsub = min(NQ, nt)
                per = -(-nt // nsub)
                s0 = 0
                while s0 < nt:
                    sn = min(per, nt - s0)
                    gi = nc.gpsimd.dma_gather(
                        out_ap=g[:, s0:s0 + sn, :], in_ap=t1_dram[:, :],
                        idxs_ap=xix[:, (t0 + s0) * 8:(t0 + s0 + sn) * 8],
                        num_idxs=sn * P, num_idxs_reg=sn * P,
                        elem_size=H, single_packet=False,
                        queue_num=pd["i"] % NQ)
                    chain_pool_dma(gi)
                    s0 += sn
                # scale whole call's tiles by per-node dinv in one DVE op
                mnb = bootg.tile([P, BG, H], BF16, tag="mn")
                dbase = dinv_pm[:, t0:t0 + nt]
                dbc = bass.AP(dbase.tensor, dbase.offset,
                              list(dbase.ap) + [[0, H]])
                nc.vector.tensor_tensor(out=mnb[:, :nt, :], in0=g[:, :nt, :],
                                        in1=dbc, op=OP.mult)
                for i in range(nt):
                    t = t0 + i
                    nq = min(P, NC - t * P)
                    if nq <= 0:
                        break
                    nc.sync.dma_start(out=tab_shard[t * P:t * P + nq, :],
                                      in_=mnb[:nq, i, :])
                    tp_ps = pro_ps.tile([P, P], BF16, tag="tp")
                    nc.tensor.transpose(out=tp_ps[:], in_=mnb[:, i, :],
                                        identity=ident_bf[:])
                    nc.scalar.activation(out=TpT[:, t * P:(t + 1) * P],
                                         in_=tp_ps[:], func=AF.Copy)
            nc.gpsimd.collective_compute(
                "AllGather", OP.bypass, replica_groups=[cores],
                ins=[tab_shard[:, :]], outs=[tab_full[:, :]])

        # ---------------- unified edge pass ----------------
        t_g = st.t_g
        t_w = st.t_w
        nt_q = st.nt_q
        # tile offset of group (w, c) and quad q in the window-major stream
        gt0 = [0] * (W * CH)
        for g in range(1, W * CH):
            gt0[g] = gt0[g - 1] + t_g[g - 1]
        qt0 = [0] * Q
        for q in range(1, Q):
            qt0[q] = qt0[q - 1] + nt_q[q - 1]

        QW = max(nt_q) * 8  # gidx cols per quad (upper bound)

        def edge_pass(layer, h_out, node_major):
            lname = f"layer{layer + 1}"
            with nc.named_scope(lname), \
                 tc.tile_pool(name=f"ix{layer}", bufs=2) as ixp, \
                 tc.tile_pool(name=f"gb{layer}", bufs=2) as gb, \
                 tc.tile_pool(name=f"bq{layer}", bufs=3) as bq, \
                 tc.tile_pool(name=f"fl{layer}", bufs=3) as fl, \
                 tc.tile_pool(name=f"eps{layer}", bufs=3, space="PSUM") as eps, \
                 tc.tile_pool(name=f"tps{layer}", bufs=2, space="PSUM") as tps, \
                 tc.tile_pool(name=f"tps2{layer}", bufs=2, space="PSUM") as tps2, \
                 tc.tile_pool(name=f"tbl{layer}", bufs=3) as tbl:

                gbuf = {}

                def issue_range(q):
                    # per-(window, chunk) gather calls; trailing -1 pad
                    # indices are skipped via the per-core count register
                    nt = nt_q[q]
                    gx = ixp.tile([P, QW], I16, tag="gx")
                    nc.sync.dma_start(
                        out=gx[:, :nt * 8],
                        in_=gidx_d[:, qt0[q] * 8:(qt0[q] + nt) * 8])
                    g = gb.tile([P, NTQ, H], BF16, tag="g")
                    for b in range(4):
                        w = q * 4 + b
                        if w >= W:
                            break
                        for c in range(CH):
                            gi_ = w * CH + c
                            tg = t_g[gi_]
                            so = gt0[gi_] - qt0[q]  # tile slot within quad
                            gcall = nc.gpsimd.dma_gather(
                                out_ap=g[:, so:so + tg, :],
                                in_ap=tab_full[c * CHN:(c + 1) * CHN, :],
                                idxs_ap=gx[:, so * 8:(so + tg) * 8],
                                num_idxs=tg * P, num_idxs_reg=tg * P,
                                elem_size=H, single_packet=False,
                                queue_num=pd["i"] % NQ)
                            chain_pool_dma(gcall, chain=False)
                    gbuf[q] = g

                def build_B(mcol, ntw):
                    # one-hot rows for a whole window group in ONE DVE op:
                    # B[p, t, j] = (iota[j] == drel[p, mcol+t])
                    Bw = bq.tile([P, NTW, WIN], BF16, tag="Bw")
                    base = meta_bf[:, mcol:mcol + ntw]
                    mb = bass.AP(base.tensor, base.offset,
                                 list(base.ap) + [[0, WIN]])
                    nc.vector.tensor_tensor(
                        out=Bw[:, :ntw, :], in0=iotaB[:, :ntw, :],
                        in1=mb, op=OP.is_equal)
                    return Bw

                def emit_tile(g, slot, Bw, wslot, qpsum, wrel, last):
                    reg = qpsum[:, wrel * WIN:(wrel + 1) * WIN]
                    if node_major:
                        nc.tensor.matmul(out=reg, lhsT=Bw[:, wslot, :],
                                         rhs=g[:, slot, :],
                                         start=False, stop=last)
                    else:
                        nc.tensor.matmul(out=reg, lhsT=g[:, slot, :],
                                         rhs=Bw[:, wslot, :],
                                         start=False, stop=last)

                def table_phase_quad(q, nxt_layer):
                    # hw for quad q of h_out -> T' rows + TpT (for next layer)
                    ncol = min(512, NCP - q * 512)
                    hw_ps = tps.tile([P, 512], F32, tag="hw")
                    nc.tensor.matmul(
                        out=hw_ps[:, :ncol],
                        lhsT=w_bf[:, nxt_layer * H:(nxt_layer + 1) * H],
                        rhs=h_out[:, q * 512:q * 512 + ncol],
                        start=True, stop=True)
                    nc.vector.tensor_tensor(
                        out=TpT[:, q * 512:q * 512 + ncol],
                        in0=hw_ps[:, :ncol],
                        in1=dinvR[:, q * 512:q * 512 + ncol], op=OP.mult)
                    for b in range(4):
                        t = q * 4 + b
                        if t >= W:
                            break
                        nq = min(P, NC - t * P)
                        if nq <= 0:
                            break
                        tp_ps = tps2.tile([P, P], BF16, tag="tr")
                        nc.tensor.transpose(
                            out=tp_ps[:], in_=TpT[:, t * P:(t + 1) * P],
                            identity=ident_bf[:])
                        stg = tbl.tile([P, P], BF16, tag="stg")
                        nc.scalar.activation(out=stg[:], in_=tp_ps[:],
                                             func=AF.Copy)
                        nc.sync.dma_start(
                            out=tab_shard[t * P:t * P + nq, :],
                            in_=stg[:nq, :])

                def pool_quad(q, pool_ps0, pool_ps1, pm, h_src):
                    for b in range(4):
                        t = q * 4 + b
                        if t >= W:
                            break
                        Bp = fl.tile([P, GS], BF16, tag="Bp")
                        nc.vector.tensor_scalar(
                            out=Bp[:], in0=iota_pool[:],
                            scalar1=pm[:, t:t + 1], scalar2=None,
                            op0=OP.is_equal)
                        blk = h_src[:, t * P:(t + 1) * P]
                        nc.tensor.matmul(out=pool_ps0[:], lhsT=Bp[:, :P],
                                         rhs=blk, start=(t == 0),
                                         stop=(t == W - 1))
                        nc.tensor.matmul(out=pool_ps1[:], lhsT=Bp[:, P:],
                                         rhs=blk, start=(t == 0),
                                         stop=(t == W - 1))

                mctr = [0]  # meta column counter (window-major, per layer)
                issue_range(0)
                for q in range(Q):
                    if q + 1 < Q:
                        issue_range(q + 1)
                    ncol = min(512, NCP - q * 512)
                    qpsum = eps.tile([P, 512], F32, tag="qp")
                    # self-loop injection (opens the accumulation group)
                    if node_major:
                        for b in range(4):
                            w = q * 4 + b
                            if w >= W:
                                break
                            nc.tensor.matmul(
                                out=qpsum[:, b * WIN:(b + 1) * WIN],
                                lhsT=TpT[:, w * WIN:(w + 1) * WIN],
                                rhs=ident_bf[:], start=(b == 0), stop=False)
                    else:
                        nc.tensor.matmul(
                            out=qpsum[:, :ncol], lhsT=ident_bf[:],
                            rhs=TpT[:, q * 512:q * 512 + ncol],
                            start=True, stop=False)
                    # edge matmuls
                    g = gbuf[q]
                    done = 0
                    for b in range(4):
                        w = q * 4 + b
                        if w >= W:
                            break
                        ntw = t_w[w]
                        Bw = build_B(mctr[0], ntw)
                        for i in range(ntw):
                            done += 1
                            emit_tile(g, done - 1, Bw, i, qpsum, b,
                                      done == nt_q[q])
                        mctr[0] += ntw
                    # flush
                    if node_major:
                        for b in range(4):
                            w = q * 4 + b
                            if w >= W:
                                break
                            nc.vector.scalar_tensor_tensor(
                                out=h_out[:, w * WIN:(w + 1) * WIN],
                                in0=qpsum[:, b * WIN:(b + 1) * WIN],
                                scalar=dinv_pm[:, w:w + 1],
                                in1=bias3R[:], op0=OP.mult, op1=OP.add)
                    else:
                        tmp = fl.tile([P, 512], BF16, tag="tmp")
                        nc.vector.tensor_tensor(
                            out=tmp[:, :ncol], in0=qpsum[:, :ncol],
                            in1=dinvR[:, q * 512:q * 512 + ncol], op=OP.mult)
                        nc.scalar.activation(
                            out=h_out[:, q * 512:q * 512 + ncol],
                            in_=tmp[:, :ncol], func=AF.Relu,
                            bias=b_cols[:, layer:layer + 1], scale=1.0)
                    # interleaved next-phase work
                    if layer < cfg.L - 1 and DEBUG_STAGE == 0:
                        table_phase_quad(q, layer + 1)
                if layer < cfg.L - 1:
                    if DEBUG_STAGE != 0:
                        for q in range(Q):
                            table_phase_quad(q, layer + 1)
                    nc.gpsimd.collective_compute(
                        "AllGather", OP.bypass, replica_groups=[cores],
                        ins=[tab_shard[:, :]], outs=[tab_full[:, :]])

        def dump_h(src_tile):
            dbg_d = nc.declare_dram_parameter("dbg", [P, NCP], F32,
                                              isOutput=True)
            with tc.tile_pool(name="dbg", bufs=2) as dbp:
                CWD = 512
                for s0 in range(0, NCP, CWD):
                    nn = min(CWD, NCP - s0)
                    dt_ = dbp.tile([P, CWD], F32, tag="d")
                    nc.vector.tensor_copy(out=dt_[:, :nn],
                                          in_=src_tile[:, s0:s0 + nn])
                    nc.sync.dma_start(out=dbg_d[:, s0:s0 + nn],
                                      in_=dt_[:, :nn])

        edge_pass(0, hT_a, node_major=False)
        if DEBUG_STAGE == 1:
            dump_h(hT_a)
        edge_pass(1, hT_b, node_major=False)
        if DEBUG_STAGE == 2:
            dump_h(hT_b)
        edge_pass(2, h3nm, node_major=True)
        if DEBUG_STAGE == 3:
            dump_h(h3nm)

        # ---------------- pooling ----------------
        with nc.named_scope("pool"), \
             tc.tile_pool(name="po", bufs=3) as po, \
             tc.tile_pool(name="po_ps", bufs=2, space="PSUM") as po_ps, \
             tc.tile_pool(name="po_acc", bufs=2, space="PSUM") as po_acc:
            pm = po.tile([P, W], F32, tag="pm")
            nc.sync.dma_start(out=pm[:], in_=pool_meta[:, :])
            gcols = po.tile([P, 2], I32, tag="gcols")
            nc.sync.dma_start(out=gcols[:], in_=gid_cols[:, :])
            recip_sb = po.tile([P, GW], F32, tag="recip")
            nc.sync.dma_start(out=recip_sb[:], in_=recip_pm[:, :])

            acc0 = po_acc.tile([P, P], F32)
            acc1 = po_acc.tile([P, P], F32)
            for t in range(W):
                Bp = po.tile([P, GS], BF16, tag="Bp")
                nc.vector.tensor_scalar(
                    out=Bp[:], in0=iota_pool[:],
                    scalar1=pm[:, t:t + 1], scalar2=None,
                    op0=OP.is_equal)
                blk = h3nm[:, t * P:(t + 1) * P]
                nc.tensor.matmul(out=acc0[:], lhsT=Bp[:, :P], rhs=blk,
                                 start=(t == 0), stop=(t == W - 1))
                nc.tensor.matmul(out=acc1[:], lhsT=Bp[:, P:], rhs=blk,
                                 start=(t == 0), stop=(t == W - 1))

            def dummy_gather():
                dz = po.tile([P, 1, P], BF16, tag="dz")
                zi = po.tile([P, 8], I16, tag="zi")
                nc.vector.memset(zi[:], 0)
                gi = nc.gpsimd.dma_gather(
                    out_ap=dz[:], in_ap=t1_dram[:, :], idxs_ap=zi[:],
                    num_idxs=P, num_idxs_reg=P, elem_size=H,
                    single_packet=False, queue_num=pd["i"] % NQ)
                chain_pool_dma(gi)

            zt = po.tile([P, P], F32, tag="zt")
            nc.vector.memset(zt[:], 0.0)
            for r0 in range(0, cfg.G + GS, P):
                nc.sync.dma_start(out=pooled_nm[r0:r0 + P, :], in_=zt[:])

            for half, acc in ((0, acc0), (1, acc1)):
                rows = po.tile([P, P], F32, tag="rows")
                nc.scalar.activation(out=rows[:], in_=acc[:], func=AF.Copy)
                while pd["i"] % NQ != 0:
                    dummy_gather()  # scatters run on queue 0: align lane
                si = nc.gpsimd.indirect_dma_start(
                    out=pooled_nm[:, :],
                    out_offset=IndirectOffsetOnAxis(
                        ap=gcols[:, half:half + 1], axis=0),
                    in_=rows[:], in_offset=None)
                chain_pool_dma(si)

            nc.gpsimd.collective_compute(
                "AllReduce", OP.add, replica_groups=[cores],
                ins=[pooled_nm[:, :]], outs=[pooled_sum[:, :]])

            for gw in range(GW):
                ot = po.tile([P, H], F32, tag="ot")
                nc.sync.dma_start(out=ot[:],
                                  in_=pooled_sum[gw * P:(gw + 1) * P, :])
                os = po.tile([P, H], F32, tag="os")
                nc.vector.tensor_scalar(
                    out=os[:], in0=ot[:], scalar1=recip_sb[:, gw:gw + 1],
                    scalar2=None, op0=OP.mult)
                nc.sync.dma_start(out=out_d[gw * P:(gw + 1) * P, :],
                                  in_=os[:])

    return nc


# --------------------------------------------------------------------------
# entry point: full inputs -> full output
# --------------------------------------------------------------------------

_CACHE = {}


def _get_compiled(cfg, st_key, st):
    if st_key not in _CACHE:
        nc = build_nc(cfg, st)
        nc.finalize()
        _CACHE[st_key] = nc
    return _CACHE[st_key]


def kernel(x, edge_index, batch, emb_table, Ws, bs):
    cfg = Cfg()  # full problem size, hardcoded
    st, in_maps = preprocess(x, edge_index, batch, emb_table, Ws, bs, cfg)
    st_key = tuple(tuple(r) for r in st.t_cw)
    nc = _get_compiled(cfg, st_key, st)

    from concourse.bass_utils import run_bass_kernel_spmd

    res = run_bass_kernel_spmd(nc, in_maps, list(range(cfg.C)))
    return np.ascontiguousarray(res.results[0]["out"])


# revision 37
# speedup vs baseline: 1.9421x; 1.0928x over previous
"""GCN embedder kernel for TRN2, 8-core SPMD (v6: 128-wide dst windows,
pure-one-hot scatter matmuls, dinv factoring, unified 3-layer edge pass).

Design
------
* Nodes sharded contiguously across C=8 cores (NC=12500 each). Edges are
  owned by the dst core. Self-loops are NOT materialized as edges.
* Normalization factored: norm(s,d) = dinv[s]*dinv[d]. dinv[src] is
  pre-multiplied into the gather table rows (T'[v] = dinv[v] * (h@W)[v]),
  dinv[dst] is applied at PSUM flush. The self-loop term
  dinv[d]^2*(h@W)[d] = dinv[d]*T'[d] is injected by an identity matmul
  of the local feature-major T' into PSUM before the edge matmuls.
* Edge pass (identical structure for all 3 layers): edges sorted by
  (src_chunk, window, dst) where window = 128 dst nodes. Per-edge-tile
  (128 edges) the one-hot B[e, drel] = is_equal(iota, drel) is ONE DVE
  tensor_scalar (or 2 scalar-engine ACTs for a fraction of tiles); one
  PE matmul accumulates into the 512-wide quad PSUM bank at the window's
  128-col offset. Layers 1-2: lhsT=messages, rhs=B -> feature-major PSUM.
  Layer 3: lhsT=B, rhs=messages -> node-major PSUM (pooling needs
  node-major and layer 3 feeds nothing else).
* Gathers: per (chunk, quad-range) - ~100 calls/layer of ~2-3K indices
  (994ns fixed GpSimd cost per SWDGE call makes small calls ruinous).
  Table rows are bf16 [N,128]=256B. Indices int16 (chunk-relative).
  The SAME index/meta streams serve all 3 layers (layer 1's table is
  per-node: T1n[v] = dinv[v]*T1[x[v]], vocab-gathered on chip).
* Table phase (interleaved into the edge pass of the previous layer at
  quad granularity): hwT = W^T @ hT (PE), T'T = hwT * dinvR (DVE),
  transpose blocks to rows (PE), DMA to DRAM shard; AllGather.
* Pooling: layer-3 node-major blocks feed is_equal(batchrel) one-hot
  matmuls directly (lhsT=Bpool half, rhs=h3 block) accumulating
  [grel<=256, H] in 2 PSUM banks; flush rows scatter by graph id
  (indirect DMA), AllReduce, scale by 1/cnt.

All structure (tile counts, call sizes) is maxed across cores so the
single SPMD program fits every core; pad slots have drel=-1 (zero
one-hot column) and index 0 (valid row).
"""

import math
from contextlib import ExitStack
from dataclasses import dataclass, field

import numpy as np

import concourse.mybir as mybir
import concourse.tile as tile
from concourse import bacc, bass
from concourse.bass import AP, IndirectOffsetOnAxis, ds
from concourse.masks import make_identity

F32 = mybir.dt.float32
BF16 = mybir.dt.bfloat16
I16 = mybir.dt.int16
I32 = mybir.dt.int32
AF = mybir.ActivationFunctionType
OP = mybir.AluOpType

P = 128  # partitions / hidden size / vocab

DEBUG_STAGE = 0  # 0=off; 1..3 = dump h after that layer


@dataclass
class Cfg:
    N: int = 100000
    E: int = 1600000
    H: int = 128
    V: int = 128
    L: int = 3
    G: int = 1024
    C: int = 8          # cores
    CH: int = 4         # gather-table chunks (int16 index limit)
    WIN: int = 128      # dst window width
    NQ: int = 4         # SWDGE queues
    SCALAR_FRAC_NUM: int = 1   # of every DEN tiles, NUM one-hots on Scalar
    SCALAR_FRAC_DEN: int = 7

    @property
    def NC(self):
        assert self.N % self.C == 0
        return self.N // self.C

    @property
    def CHN(self):
        assert self.N % self.CH == 0
        return self.N // self.CH

    @property
    def W(self):  # 128-wide dst windows per core
        return math.ceil(self.NC / self.WIN)

    @property
    def Q(self):  # 512-wide quads (ranges) per core
        return math.ceil(self.W / 4)

    @property
    def NCP(self):
        return self.W * self.WIN

    @property
    def GSPAN(self):
        return 256


@dataclass
class Structure:
    # t_cw[c][w]: edge tiles for (chunk c, window w), maxed across cores
    t_cw: list = field(default_factory=list)

    @property
    def T(self):
        return sum(sum(r) for r in self.t_cw)


def preprocess(x, edge_index, batch, emb_table, Ws, bs, cfg: Cfg):
    """Host-side (index-only) preprocessing."""
    N, E, C, CH = cfg.N, cfg.E, cfg.C, cfg.CH
    NC, CHN, W, Q, WIN = cfg.NC, cfg.CHN, cfg.W, cfg.Q, cfg.WIN

    x = np.asarray(x).astype(np.int64)
    edge_index = np.asarray(edge_index).astype(np.int64)
    batch = np.asarray(batch).astype(np.int64)

    src, dst = edge_index[0], edge_index[1]
    deg = (np.bincount(dst, minlength=N) + 1).astype(np.float32)  # + self
    dinv = (1.0 / np.sqrt(deg)).astype(np.float32)

    owner = dst // NC
    per_core = []
    for c in range(C):
        m = owner == c
        s_c = src[m]
        d_c = dst[m] - c * NC
        w_c = d_c // WIN
        ck = s_c // CHN
        o = np.lexsort((d_c, ck, w_c))  # window-major, then chunk, then dst
        per_core.append(dict(s=(s_c[o] % CHN), drel=(d_c[o] % WIN),
                             grp=(w_c[o] * CH + ck[o])))

    # tiles per (window, chunk) group, maxed across cores
    NG = W * CH
    t_g = np.zeros(NG, dtype=np.int64)
    for c in range(C):
        cnt = np.bincount(per_core[c]["grp"], minlength=NG)
        t_g = np.maximum(t_g, -(-cnt // P))
    t_g = np.maximum(t_g, 1)
    st = Structure(t_cw=[list(map(int, t_g))])
    T = int(t_g.sum())
    gt0 = np.concatenate([[0], np.cumsum(t_g)[:-1]])  # tile offset per group

    def wrap(arr):
        # [n] int16 -> [128, n//16] wrapped in 16 partitions, tiled x8
        wr = arr.reshape(-1, 16).T
        return np.tile(wr, (8, 1))

    in_maps = []
    for c in range(C):
        pc = per_core[c]
        cnt = np.bincount(pc["grp"], minlength=NG)
        starts = np.concatenate([[0], np.cumsum(cnt)[:-1]])

        # meta + idx stream, both in (window, chunk) consumption order;
        # pads: drel=-1 (zero one-hot col), idx=-1 (trailing - skipped by
        # the gather via the per-core real-count register)
        meta = np.full((P, T), -1.0, dtype=np.float32)
        idxs = np.zeros(T * P, dtype=np.int16)
        for g in range(NG):
            n = int(cnt[g])
            sl = slice(int(starts[g]), int(starts[g]) + n)
            ii = np.arange(n)
            t0 = int(gt0[g])
            meta[ii % P, t0 + ii // P] = pc["drel"][sl]
            idxs[t0 * P + ii] = pc["s"][sl].astype(np.int16)
        gidx = np.zeros((P, T * 8), dtype=np.int16)
        for g in range(NG):
            t0, nt = int(gt0[g]), int(t_g[g])
            gidx[:, t0 * 8:(t0 + nt) * 8] = wrap(idxs[t0 * P:(t0 + nt) * P])
        cnts = np.broadcast_to(cnt.astype(np.int32)[None, :],
                               (P, NG)).copy()

        # vocab gather indices for the layer-1 table (node-major, pad->0)
        nodes = np.arange(cfg.NCP) + c * NC
        valid = nodes < (c + 1) * NC
        xl = np.where(valid, x[np.minimum(nodes, N - 1)], 0)
        xidx = wrap(xl.astype(np.int16))  # [128, W*8]

        dloc = np.where(valid, dinv[np.minimum(nodes, N - 1)], 0.0)
        dinv_pm = dloc.reshape(W, P).T.copy().astype(np.float32)  # [128, W]
        dinvR = np.broadcast_to(dloc[None, :], (P, cfg.NCP)).astype(np.float32)

        bias3R = np.broadcast_to(np.asarray(bs)[2][None, :],
                                 (P, cfg.H)).astype(np.float32).copy()

        # pooling metadata (as baseline)
        bvals = np.where(valid, batch[np.minimum(nodes, N - 1)], -1)
        gmin = int(batch[c * NC])
        gmax = int(batch[min((c + 1) * NC, N) - 1])
        assert gmax - gmin < cfg.GSPAN, (c, gmin, gmax)
        brel = np.where(valid, bvals - gmin, -1).astype(np.float32)
        pool_meta = brel.reshape(W, P).T.copy()  # [128, W]
        gid_rows = gmin + np.arange(cfg.GSPAN)
        gid_rows = np.where(gid_rows < cfg.G, gid_rows,
                            cfg.G + np.arange(cfg.GSPAN) % 256).astype(np.int32)
        gid_cols = gid_rows.reshape(2, P).T.copy()  # [128, 2]

        cnts = np.bincount(batch, minlength=cfg.G).astype(np.float32)
        recip = 1.0 / np.maximum(cnts, 1.0)
        recip_pm = recip.reshape(cfg.G // P, P).T.copy()

        in_maps.append({
            "meta": meta, "gidx": gidx,
            "xidx": np.ascontiguousarray(xidx),
            "dinv_pm": dinv_pm, "dinvR": np.ascontiguousarray(dinvR),
            "bias3R": bias3R,
            "pool_meta": pool_meta, "gid_cols": gid_cols,
            "recip_pm": recip_pm,
            "emb": np.asarray(emb_table, dtype=np.float32),
            "Ws": np.asarray(Ws, dtype=np.float32),
            "bs": np.asarray(bs, dtype=np.float32),
        })

    # per-window tiles and per-quad totals
    t_w = t_g.reshape(W, CH).sum(axis=1)
    nt_q = [int(t_w[4 * q:min(4 * q + 4, W)].sum()) for q in range(Q)]
    st.t_g = [int(v) for v in t_g]
    st.t_w = [int(v) for v in t_w]
    st.nt_q = nt_q
    st.NTQ = max(nt_q)
    st.NTW = int(t_w.max())
    st.NG = NG
    return st, in_maps


# --------------------------------------------------------------------------
# device program
# --------------------------------------------------------------------------

def build_nc(cfg: Cfg, st: Structure):
    N, H, C, CH, W, Q = cfg.N, cfg.H, cfg.C, cfg.CH, cfg.W, cfg.Q
    NC, CHN, NCP, WIN = cfg.NC, cfg.CHN, cfg.NCP, cfg.WIN
    T = st.T
    NTQ = st.NTQ
    GS = cfg.GSPAN
    GW = cfg.G // P
    NQ = cfg.NQ

    nc = bacc.Bacc(None, num_devices=C, num_swdge_queues=NQ)
    cores = list(range(C))

    # ---- external I/O ----
    meta_d = nc.declare_dram_parameter("meta", [P, T], F32, isOutput=False)
    gidx_d = nc.declare_dram_parameter("gidx", [P, T * 8], I16, isOutput=False)
    xidx_d = nc.declare_dram_parameter("xidx", [P, W * 8], I16, isOutput=False)
    dinv_pm_d = nc.declare_dram_parameter("dinv_pm", [P, W], F32, isOutput=False)
    dinvR_d = nc.declare_dram_parameter("dinvR", [P, NCP], F32, isOutput=False)
    bias3R_d = nc.declare_dram_parameter("bias3R", [P, H], F32, isOutput=False)
    pool_meta = nc.declare_dram_parameter("pool_meta", [P, W], F32, isOutput=False)
    gid_cols = nc.declare_dram_parameter("gid_cols", [P, 2], I32, isOutput=False)
    recip_pm = nc.declare_dram_parameter("recip_pm", [P, GW], F32, isOutput=False)
    emb_d = nc.declare_dram_parameter("emb", [P, H], F32, isOutput=False)
    Ws_d = nc.declare_dram_parameter("Ws", [cfg.L, H, H], F32, isOutput=False)
    bs_d = nc.declare_dram_parameter("bs", [cfg.L, H], F32, isOutput=False)
    out_d = nc.declare_dram_parameter("out", [cfg.G, H], F32, isOutput=True)

    # ---- internal DRAM ----
    t1_dram = nc.dram_tensor("t1_tab", [cfg.V, H], BF16)
    tab_shard = nc.dram_tensor("tab_shard", [NC, H], BF16)
    tab_full = nc.dram_tensor("tab_full", [N, H], BF16, addr_space="Shared")
    pooled_nm = nc.dram_tensor("pooled_nm", [cfg.G + GS, H], F32)
    pooled_sum = nc.dram_tensor("pooled_sum", [cfg.G + GS, H], F32,
                                addr_space="Shared")

    from concourse.tile import add_dep_helper
    pd = {"i": 0, "last": None}

    def chain_pool_dma(inst, chain=True):
        if chain and pd["last"] is not None:
            add_dep_helper(inst.ins, pd["last"].ins, sync=False,
                           reason="pool-dma queue/lane parity order")
        pd["last"] = inst
        pd["i"] += 1

    with tile.TileContext(nc) as tc, ExitStack() as ctx:
        const = ctx.enter_context(tc.tile_pool(name="const", bufs=1))
        hpool = ctx.enter_context(tc.tile_pool(name="hbuf", bufs=1))

        ident = const.tile([P, P], F32)
        make_identity(nc, ident[:])
        ident_bf = const.tile([P, P], BF16)
        make_identity(nc, ident_bf[:])
        iota_i = const.tile([P, 512], I32)
        nc.gpsimd.iota(iota_i[:], pattern=[[1, 512]], base=0,
                       channel_multiplier=0)
        iota_pool = const.tile([P, GS], BF16)
        nc.vector.tensor_copy(out=iota_pool[:], in_=iota_i[:, :GS])
        # repeating 0..WIN-1 pattern, one block per tile of a window group
        NTW = st.NTW
        iotaB_i = const.tile([P, NTW, WIN], I32)
        nc.gpsimd.iota(iotaB_i[:], pattern=[[0, NTW], [1, WIN]], base=0,
                       channel_multiplier=0)
        iotaB = const.tile([P, NTW, WIN], BF16)
        nc.vector.tensor_copy(out=iotaB[:], in_=iotaB_i[:])

        b_cols = const.tile([P, cfg.L], F32)
        for l in range(cfg.L):
            nc.sync.dma_start(out=b_cols[:, l:l + 1], in_=bs_d[l, :, None])
        w_bf = const.tile([P, cfg.L * H], BF16, tag="w_bf")
        with tc.tile_pool(name="wload", bufs=2) as wl:
            for l in range(cfg.L):
                wt = wl.tile([P, H], F32, tag="wt")
                nc.sync.dma_start(out=wt[:], in_=Ws_d[l])
                nc.vector.tensor_copy(out=w_bf[:, l * H:(l + 1) * H], in_=wt[:])
        bias3R = const.tile([P, H], F32)
        nc.sync.dma_start(out=bias3R[:], in_=bias3R_d[:, :])
        dinv_pm = const.tile([P, W], F32)
        nc.sync.dma_start(out=dinv_pm[:], in_=dinv_pm_d[:, :])

        # resident meta (bf16: drel in 0..127 and -1 are exact) + dinvR (bf16)
        meta_bf = const.tile([P, T], BF16, tag="meta_bf")
        with tc.tile_pool(name="mld", bufs=2) as mld:
            MC = 1024
            for s0 in range(0, T, MC):
                nn = min(MC, T - s0)
                mt_ = mld.tile([P, MC], F32, tag="m")
                nc.sync.dma_start(out=mt_[:, :nn], in_=meta_d[:, s0:s0 + nn])
                nc.vector.tensor_copy(out=meta_bf[:, s0:s0 + nn],
                                      in_=mt_[:, :nn])
        dinvR = const.tile([P, NCP], BF16, tag="dinvR")
        with tc.tile_pool(name="dld", bufs=2) as dld:
            for s0 in range(0, NCP, 512):
                nn = min(512, NCP - s0)
                dt_ = dld.tile([P, 512], F32, tag="d")
                nc.sync.dma_start(out=dt_[:, :nn], in_=dinvR_d[:, s0:s0 + nn])
                nc.vector.tensor_copy(out=dinvR[:, s0:s0 + nn],
                                      in_=dt_[:, :nn])

        hT_a = hpool.tile([P, NCP], BF16)     # feature-major h (layers 1,2)
        hT_b = hpool.tile([P, NCP], BF16)
        TpT = hpool.tile([P, NCP], BF16)      # feature-major local T'
        h3nm = hT_a                           # layer-3 out (node-major) aliases
        #                                       layer-1 h (dead by then)

        # ---------------- layer-1 table: T1n[v] = dinv[v]*T1[x[v]] ----------
        with nc.named_scope("boot"), \
             tc.tile_pool(name="pro", bufs=2) as pro, \
             tc.tile_pool(name="pro_ps", bufs=2, space="PSUM") as pro_ps, \
             tc.tile_pool(name="bootg", bufs=2) as bootg, \
             tc.tile_pool(name="bootix", bufs=1) as bootix:
            emb_sb = pro.tile([P, H], F32, tag="emb")
            nc.sync.dma_start(out=emb_sb[:], in_=emb_d[:, :])
            w1_sb = pro.tile([P, H], F32, tag="w1")
            nc.sync.dma_start(out=w1_sb[:], in_=Ws_d[0])
            embT_ps = pro_ps.tile([P, P], F32)
            nc.tensor.transpose(out=embT_ps[:], in_=emb_sb[:], identity=ident[:])
            embT = pro.tile([P, P], F32, tag="embT")
            nc.vector.tensor_copy(out=embT[:], in_=embT_ps[:])
            t1t_ps = pro_ps.tile([P, P], F32)
            nc.tensor.matmul(out=t1t_ps[:], lhsT=w1_sb[:], rhs=embT[:],
                             start=True, stop=True)
            t1t = pro.tile([P, P], F32, tag="t1t")
            nc.vector.tensor_copy(out=t1t[:], in_=t1t_ps[:])
            t1nm_ps = pro_ps.tile([P, P], F32)
            nc.tensor.transpose(out=t1nm_ps[:], in_=t1t[:], identity=ident[:])
            t1nm = pro.tile([P, P], BF16, tag="t1nm")
            nc.vector.tensor_copy(out=t1nm[:], in_=t1nm_ps[:])
            nc.sync.dma_start(out=t1_dram[:, :], in_=t1nm[:])

            # vocab gather (node-major), scale by dinv, rows -> tab_shard,
            # transpose -> TpT
            xix = bootix.tile([P, W * 8], I16, tag="xix")
            nc.sync.dma_start(out=xix[:], in_=xidx_d[:, :])
            BG = 14  # tiles per vocab-gather call
            for t0 in range(0, W, BG):
                nt = min(BG, W - t0)
                g = bootg.tile([P, BG, H], BF16, tag="vg")
                nsub = min(NQ, nt)
                per = -(-nt // nsub)
                s0 = 0
                while s0 < nt:
                    sn = min(per, nt - s0)
                    gi = nc.gpsimd.dma_gather(
                        out_ap=g[:, s0:s0 + sn, :], in_ap=t1_dram[:, :],
                        idxs_ap=xix[:, (t0 + s0) * 8:(t0 + s0 + sn) * 8],
                        num_idxs=sn * P, num_idxs_reg=sn * P,
                        elem_size=H, single_packet=True,
                        queue_num=pd["i"] % NQ)
                    chain_pool_dma(gi)
                    s0 += sn
                # scale whole call's tiles by per-node dinv in one DVE op
                mnb = bootg.tile([P, BG, H], BF16, tag="mn")
                dbase = dinv_pm[:, t0:t0 + nt]
                dbc = bass.AP(dbase.tensor, dbase.offset,
                              list(dbase.ap) + [[0, H]])
                nc.vector.tensor_tensor(out=mnb[:, :nt, :], in0=g[:, :nt, :],
                                        in1=dbc, op=OP.mult)
                for i in range(nt):
                    t = t0 + i
                    nq = min(P, NC - t * P)
                    if nq <= 0:
                        break
                    nc.sync.dma_start(out=tab_shard[t * P:t * P + nq, :],
                                      in_=mnb[:nq, i, :])
                    tp_ps = pro_ps.tile([P, P], BF16, tag="tp")
                    nc.tensor.transpose(out=tp_ps[:], in_=mnb[:, i, :],
                                        identity=ident_bf[:])
                    nc.scalar.activation(out=TpT[:, t * P:(t + 1) * P],
                                         in_=tp_ps[:], func=AF.Copy)
            nc.gpsimd.collective_compute(
                "AllGather", OP.bypass, replica_groups=[cores],
                ins=[tab_shard[:, :]], outs=[tab_full[:, :]])

        # ---------------- unified edge pass ----------------
        t_g = st.t_g
        t_w = st.t_w
        nt_q = st.nt_q
        # tile offset of group (w, c) and quad q in the window-major stream
        gt0 = [0] * (W * CH)
        for g in range(1, W * CH):
            gt0[g] = gt0[g - 1] + t_g[g - 1]
        qt0 = [0] * Q
        for q in range(1, Q):
            qt0[q] = qt0[q - 1] + nt_q[q - 1]

        QW = max(nt_q) * 8  # gidx cols per quad (upper bound)

        def edge_pass(layer, h_out, node_major):
            lname = f"layer{layer + 1}"
            with nc.named_scope(lname), \
                 tc.tile_pool(name=f"ix{layer}", bufs=2) as ixp, \
                 tc.tile_pool(name=f"gb{layer}", bufs=2) as gb, \
                 tc.tile_pool(name=f"bq{layer}", bufs=3) as bq, \
                 tc.tile_pool(name=f"fl{layer}", bufs=3) as fl, \
                 tc.tile_pool(name=f"eps{layer}", bufs=3, space="PSUM") as eps, \
                 tc.tile_pool(name=f"tps{layer}", bufs=2, space="PSUM") as tps, \
                 tc.tile_pool(name=f"tps2{layer}", bufs=2, space="PSUM") as tps2, \
                 tc.tile_pool(name=f"tbl{layer}", bufs=3) as tbl:

                gbuf = {}

                def issue_range(q):
                    # per-(window, chunk) gather calls; trailing -1 pad
                    # indices are skipped via the per-core count register
                    nt = nt_q[q]
                    gx = ixp.tile([P, QW], I16, tag="gx")
                    nc.sync.dma_start(
                        out=gx[:, :nt * 8],
                        in_=gidx_d[:, qt0[q] * 8:(qt0[q] + nt) * 8])
                    g = gb.tile([P, NTQ, H], BF16, tag="g")
                    for b in range(4):
                        w = q * 4 + b
                        if w >= W:
                            break
                        for c in range(CH):
                            gi_ = w * CH + c
                            tg = t_g[gi_]
                            so = gt0[gi_] - qt0[q]  # tile slot within quad
                            gcall = nc.gpsimd.dma_gather(
                                out_ap=g[:, so:so + tg, :],
                                in_ap=tab_full[c * CHN:(c + 1) * CHN, :],
                                idxs_ap=gx[:, so * 8:(so + tg) * 8],
                                num_idxs=tg * P, num_idxs_reg=tg * P,
                                elem_size=H, single_packet=True,
                                queue_num=pd["i"] % NQ)
                            chain_pool_dma(gcall, chain=False)
                    gbuf[q] = g

                def build_B(mcol, ntw):
                    # one-hot rows for a whole window group in ONE DVE op:
                    # B[p, t, j] = (iota[j] == drel[p, mcol+t])
                    Bw = bq.tile([P, NTW, WIN], BF16, tag="Bw")
                    base = meta_bf[:, mcol:mcol + ntw]
                    mb = bass.AP(base.tensor, base.offset,
                                 list(base.ap) + [[0, WIN]])
                    nc.vector.tensor_tensor(
                        out=Bw[:, :ntw, :], in0=iotaB[:, :ntw, :],
                        in1=mb, op=OP.is_equal)
                    return Bw

                def emit_tile(g, slot, Bw, wslot, qpsum, wrel, last):
                    reg = qpsum[:, wrel * WIN:(wrel + 1) * WIN]
                    if node_major:
                        nc.tensor.matmul(out=reg, lhsT=Bw[:, wslot, :],
                                         rhs=g[:, slot, :],
                                         start=False, stop=last)
                    else:
                        nc.tensor.matmul(out=reg, lhsT=g[:, slot, :],
                                         rhs=Bw[:, wslot, :],
                                         start=False, stop=last)

                def table_phase_quad(q, nxt_layer):
                    # hw for quad q of h_out -> T' rows + TpT (for next layer)
                    ncol = min(512, NCP - q * 512)
                    hw_ps = tps.tile([P, 512], F32, tag="hw")
                    nc.tensor.matmul(
                        out=hw_ps[:, :ncol],
                        lhsT=w_bf[:, nxt_layer * H:(nxt_layer + 1) * H],
                        rhs=h_out[:, q * 512:q * 512 + ncol],
                        start=True, stop=True)
                    nc.vector.tensor_tensor(
                        out=TpT[:, q * 512:q * 512 + ncol],
                        in0=hw_ps[:, :ncol],
                        in1=dinvR[:, q * 512:q * 512 + ncol], op=OP.mult)
                    for b in range(4):
                        t = q * 4 + b
                        if t >= W:
                            break
                        nq = min(P, NC - t * P)
                        if nq <= 0:
                            break
                        tp_ps = tps2.tile([P, P], BF16, tag="tr")
                        nc.tensor.transpose(
                            out=tp_ps[:], in_=TpT[:, t * P:(t + 1) * P],
                            identity=ident_bf[:])
                        stg = tbl.tile([P, P], BF16, tag="stg")
                        nc.scalar.activation(out=stg[:], in_=tp_ps[:],
                                             func=AF.Copy)
                        nc.sync.dma_start(
                            out=tab_shard[t * P:t * P + nq, :],
                            in_=stg[:nq, :])

                def pool_quad(q, pool_ps0, pool_ps1, pm, h_src):
                    for b in range(4):
                        t = q * 4 + b
                        if t >= W:
                            break
                        Bp = fl.tile([P, GS], BF16, tag="Bp")
                        nc.vector.tensor_scalar(
                            out=Bp[:], in0=iota_pool[:],
                            scalar1=pm[:, t:t + 1], scalar2=None,
                            op0=OP.is_equal)
                        blk = h_src[:, t * P:(t + 1) * P]
                        nc.tensor.matmul(out=pool_ps0[:], lhsT=Bp[:, :P],
                                         rhs=blk, start=(t == 0),
                                         stop=(t == W - 1))
                        nc.tensor.matmul(out=pool_ps1[:], lhsT=Bp[:, P:],
                                         rhs=blk, start=(t == 0),
                                         stop=(t == W - 1))

                mctr = [0]  # meta column counter (window-major, per layer)
                issue_range(0)
                for q in range(Q):
                    if q + 1 < Q:
                        issue_range(q + 1)
                    ncol = min(512, NCP - q * 512)
                    qpsum = eps.tile([P, 512], F32, tag="qp")
                    # self-loop injection (opens the accumulation group)
                    if node_major:
                        for b in range(4):
                            w = q * 4 + b
                            if w >= W:
                                break
                            nc.tensor.matmul(
                                out=qpsum[:, b * WIN:(b + 1) * WIN],
                                lhsT=TpT[:, w * WIN:(w + 1) * WIN],
                                rhs=ident_bf[:], start=(b == 0), stop=False)
                    else:
                        nc.tensor.matmul(
                            out=qpsum[:, :ncol], lhsT=ident_bf[:],
                            rhs=TpT[:, q * 512:q * 512 + ncol],
                            start=True, stop=False)
                    # edge matmuls
                    g = gbuf[q]
                    done = 0
                    for b in range(4):
                        w = q * 4 + b
                        if w >= W:
                            break
                        ntw = t_w[w]
                        Bw = build_B(mctr[0], ntw)
                        for i in range(ntw):
                            done += 1
                            emit_tile(g, done - 1, Bw, i, qpsum, b,
                                      done == nt_q[q])
                        mctr[0] += ntw
                    # flush
                    if node_major:
                        for b in range(4):
                            w = q * 4 + b
                            if w >= W:
                                break
                            nc.vector.scalar_tensor_tensor(
                                out=h_out[:, w * WIN:(w + 1) * WIN],
                                in0=qpsum[:, b * WIN:(b + 1) * WIN],
                                scalar=dinv_pm[:, w:w + 1],
                                in1=bias3R[:], op0=OP.mult, op1=OP.add)
                    else:
                        tmp = fl.tile([P, 512], BF16, tag="tmp")
                        nc.vector.tensor_tensor(
                            out=tmp[:, :ncol], in0=qpsum[:, :ncol],
                            in1=dinvR[:, q * 512:q * 512 + ncol], op=OP.mult)
                        nc.scalar.activation(
                            out=h_out[:, q * 512:q * 512 + ncol],
                            in_=tmp[:, :ncol], func=AF.Relu,
                            bias=b_cols[:, layer:layer + 1], scale=1.0)
                    # interleaved next-phase work
                    if layer < cfg.L - 1 and DEBUG_STAGE == 0:
                        table_phase_quad(q, layer + 1)
                if layer < cfg.L - 1:
                    if DEBUG_STAGE != 0:
                        for q in range(Q):
                            table_phase_quad(q, layer + 1)
                    nc.gpsimd.collective_compute(
                        "AllGather", OP.bypass, replica_groups=[cores],
                        ins=[tab_shard[:, :]], outs=[tab_full[:, :]])

        def dump_h(src_tile):
            dbg_d = nc.declare_dram_parameter("dbg", [P, NCP], F32,
                                              isOutput=True)
            with tc.tile_pool(name="dbg", bufs=2) as dbp:
                CWD = 512
                for s0 in range(0, NCP, CWD):
                    nn = min(CWD, NCP - s0)
                    dt_ = dbp.tile([P, CWD], F32, tag="d")
                    nc.vector.tensor_copy(out=dt_[:, :nn],
                                          in_=src_tile[:, s0:s0 + nn])
                    nc.sync.dma_start(out=dbg_d[:, s0:s0 + nn],
                                      in_=dt_[:, :nn])

        edge_pass(0, hT_a, node_major=False)
        if DEBUG_STAGE == 1:
            dump_h(hT_a)
        edge_pass(1, hT_b, node_major=False)
        if DEBUG_STAGE == 2:
            dump_h(hT_b)
        edge_pass(2, h3nm, node_major=True)
        if DEBUG_STAGE == 3:
            dump_h(h3nm)

        # ---------------- pooling ----------------
        with nc.named_scope("pool"), \
             tc.tile_pool(name="po", bufs=3) as po, \
             tc.tile_pool(name="po_ps", bufs=2, space="PSUM") as po_ps, \
             tc.tile_pool(name="po_acc", bufs=2, space="PSUM") as po_acc:
            pm = po.tile([P, W], F32, tag="pm")
            nc.sync.dma_start(out=pm[:], in_=pool_meta[:, :])
            gcols = po.tile([P, 2], I32, tag="gcols")
            nc.sync.dma_start(out=gcols[:], in_=gid_cols[:, :])
            recip_sb = po.tile([P, GW], F32, tag="recip")
            nc.sync.dma_start(out=recip_sb[:], in_=recip_pm[:, :])

            acc0 = po_acc.tile([P, P], F32)
            acc1 = po_acc.tile([P, P], F32)
            for t in range(W):
                Bp = po.tile([P, GS], BF16, tag="Bp")
                nc.vector.tensor_scalar(
                    out=Bp[:], in0=iota_pool[:],
                    scalar1=pm[:, t:t + 1], scalar2=None,
                    op0=OP.is_equal)
                blk = h3nm[:, t * P:(t + 1) * P]
                nc.tensor.matmul(out=acc0[:], lhsT=Bp[:, :P], rhs=blk,
                                 start=(t == 0), stop=(t == W - 1))
                nc.tensor.matmul(out=acc1[:], lhsT=Bp[:, P:], rhs=blk,
                                 start=(t == 0), stop=(t == W - 1))

            def dummy_gather():
                dz = po.tile([P, 1, P], BF16, tag="dz")
                zi = po.tile([P, 8], I16, tag="zi")
                nc.vector.memset(zi[:], 0)
                gi = nc.gpsimd.dma_gather(
                    out_ap=dz[:], in_ap=t1_dram[:, :], idxs_ap=zi[:],
                    num_idxs=P, num_idxs_reg=P, elem_size=H,
                    single_packet=True, queue_num=pd["i"] % NQ)
                chain_pool_dma(gi)

            zt = po.tile([P, P], F32, tag="zt")
            nc.vector.memset(zt[:], 0.0)
            for r0 in range(0, cfg.G + GS, P):
                nc.sync.dma_start(out=pooled_nm[r0:r0 + P, :], in_=zt[:])

            for half, acc in ((0, acc0), (1, acc1)):
                rows = po.tile([P, P], F32, tag="rows")
                nc.scalar.activation(out=rows[:], in_=acc[:], func=AF.Copy)
                while pd["i"] % NQ != 0:
                    dummy_gather()  # scatters run on queue 0: align lane
                si = nc.gpsimd.indirect_dma_start(
                    out=pooled_nm[:, :],
                    out_offset=IndirectOffsetOnAxis(
                        ap=gcols[:, half:half + 1], axis=0),
                    in_=rows[:], in_offset=None)
                chain_pool_dma(si)

            nc.gpsimd.collective_compute(
                "AllReduce", OP.add, replica_groups=[cores],
                ins=[pooled_nm[:, :]], outs=[pooled_sum[:, :]])

            for gw in range(GW):
                ot = po.tile([P, H], F32, tag="ot")
                nc.sync.dma_start(out=ot[:],
                                  in_=pooled_sum[gw * P:(gw + 1) * P, :])
                os = po.tile([P, H], F32, tag="os")
                nc.vector.tensor_scalar(
                    out=os[:], in0=ot[:], scalar1=recip_sb[:, gw:gw + 1],
                    scalar2=None, op0=OP.mult)
                nc.sync.dma_start(out=out_d[gw * P:(gw + 1) * P, :],
                                  in_=os[:])

    return nc


# --------------------------------------------------------------------------
# entry point: full inputs -> full output
# --------------------------------------------------------------------------

_CACHE = {}


def _get_compiled(cfg, st_key, st):
    if st_key not in _CACHE:
        nc = build_nc(cfg, st)
        nc.finalize()
        _CACHE[st_key] = nc
    return _CACHE[st_key]


def kernel(x, edge_index, batch, emb_table, Ws, bs):
    cfg = Cfg()  # full problem size, hardcoded
    st, in_maps = preprocess(x, edge_index, batch, emb_table, Ws, bs, cfg)
    st_key = tuple(tuple(r) for r in st.t_cw)
    nc = _get_compiled(cfg, st_key, st)

    from concourse.bass_utils import run_bass_kernel_spmd

    res = run_bass_kernel_spmd(nc, in_maps, list(range(cfg.C)))
    return np.ascontiguousarray(res.results[0]["out"])
